# revision 2
# baseline (speedup 1.0000x reference)
"""Trainium2 Bass kernel for GQA attention (B=1, S=2048, D=4096, H=32, H_KV=8, HD=128).

Sharding: tensor-parallel over heads, 8 cores; core c owns Q heads 4c..4c+3 and
KV head c.  Each core computes a partial [S, D] output (wo row-shard); the host
sums the partials.

Per-core kernel v2 (fp8-DoubleRow compensated projections):
  - QKV projection and the wo output projection run as fp8e4m3 DoubleRow
    matmuls (cost-model 0.5 cycles/row, 256-deep contraction per
    instruction).  Full fp32-level accuracy is recovered with a hi/lo
    split: w = w_hi + w_lo, x = x_hi + x_lo (host-side, power-of-2
    rescaled so both splits stay in fp8's normal range), computing
    w_hi@x_hi (chunk-paired) + (w_hi@x_lo + w_lo@x_hi) (slot-paired in a
    single DoubleRow op).  The dropped w_lo@x_lo term is ~1e-3 relative.
  - All scale factors fold into free spots: the RoPE cos/sin tables carry
    the score scale, the softmax-denominator ones-matrix carries the V
    rescale, and the host divides the summed output by a single constant.
  - QKV accumulates entirely in PSUM (6 banks), no SBUF folding; weights
    and x stream once (fp8, half traffic).
  - Attention: flash-style transposed scores (fp32r), causal block skip,
    softmax denominator via a 4.0-matrix matmul (fp32r); normalization and
    the attout hi/lo fp8 split happen on DVE/ACT.
  - Output staged to bf16 in SBUF, DMA'd out, host sums in fp32.
  - PE phase order: p0 p1 a0 p2 a1 w0 p3 a2 w1 a3 w2 w3 so RoPE (DVE),
    exp (ACT), attout finalization and output DMA all hide under PE work.
"""

import math
import os
import sys
import time

import numpy as np
import ml_dtypes

E4 = ml_dtypes.float8_e4m3fn


def _log(msg):
    if os.environ.get("KERNEL_QUIET"):
        return
    print(f"[kernel {time.strftime('%H:%M:%S')}] {msg}", file=sys.stderr, flush=True)


import concourse.bass as bass
import concourse.tile as tile
from concourse import bacc, mybir
from concourse.bass_utils import run_bass_kernel_spmd

S, D = 2048, 4096
H, H_KV, HD = 32, 8, 128
NCORES = 8
HPC = H // NCORES            # 4 Q heads per core
NSLAB = 6                    # q0..q3, k, v slabs of 128 cols
SQ = 512
NSQ = S // SQ                # 4
NKT = S // 128               # 16 key tiles
NDC = D // 128               # 32 contraction chunks
GRP = 8                      # chunks per DMA group
NG = NDC // GRP              # 4 groups
F32 = mybir.dt.float32
F32R = mybir.dt.float32r
F8 = mybir.dt.float8e4
BF16 = mybir.dt.bfloat16
DR = mybir.MatmulPerfMode.DoubleRow
Exp = mybir.ActivationFunctionType.Exp

WSC = 64.0                       # weight rescale for fp8 range
BETA = 1.0 / (WSC * 128.0 ** 0.25)   # rope cos/sin scale (per q and k)
CONES = 4.0                      # denominator matrix value -> attout = 16x true
OUT_DIV = (WSC / CONES) * WSC    # host divides summed output by this (1024)

_NC_CACHE = {}


def _build_nc():
    nc = bacc.Bacc(
        "TRN2", target_bir_lowering=False, debug=False, enable_asserts=False
    )
    w8 = nc.dram_tensor("w8", [D, 2 * NSLAB * 128], F8, kind="ExternalInput")
    # s-chunk-major: [NSQ * D, (lo, hi) * SQ] so each (g, s) load is 3-dim
    x8 = nc.dram_tensor("x8", [NSQ * D, 2 * SQ], F8, kind="ExternalInput")
    wor8 = nc.dram_tensor("wor8", [128, HPC * 2 * D], F8, kind="ExternalInput")
    cosq = nc.dram_tensor("cosq", [64, S], BF16, kind="ExternalInput")
    sinq = nc.dram_tensor("sinq", [64, S], BF16, kind="ExternalInput")
    diagm = nc.dram_tensor("diagm", [128, 4 * SQ], BF16, kind="ExternalInput")
    ones4 = nc.dram_tensor("ones4", [128, 128], F32R, kind="ExternalInput")
    identd = nc.dram_tensor("identd", [128, 128], F32R, kind="ExternalInput")
    out = nc.dram_tensor("out", [S, D], BF16, kind="ExternalOutput")

    _log("emitting IR")
    with tile.TileContext(nc) as tc:
        _emit(tc, w8, x8, wor8, cosq, sinq, diagm, ones4, identd, out)
    _log("bacc compile")
    nc.compile()
    _log("bass module ready")
    return nc


def _emit(tc, w8, x8, wor8, cosq, sinq, diagm, ones4, identd, out):
    from contextlib import ExitStack

    nc = tc.nc
    WROW = 2 * NSLAB * 128       # 1536 fp8 cols per d-chunk of w8
    with ExitStack() as ctx:
        const = ctx.enter_context(tc.tile_pool(name="const", bufs=1))
        wres = ctx.enter_context(tc.tile_pool(name="wres", bufs=1))
        slabs = ctx.enter_context(tc.tile_pool(name="slabs", bufs=1))
        xpool = ctx.enter_context(tc.tile_pool(name="xpool", bufs=2))
        vtmp = ctx.enter_context(tc.tile_pool(name="vtmp", bufs=1))
        ptpool = ctx.enter_context(tc.tile_pool(name="ptpool", bufs=4))
        # separate rope scratch rings per engine: a shared ring would make
        # the Pool rope ops wait on DVE's tile releases (serializing them)
        tmppool = ctx.enter_context(tc.tile_pool(name="tmppool", bufs=4))
        tmppoolp = ctx.enter_context(tc.tile_pool(name="tmppoolp", bufs=4))
        recpool = ctx.enter_context(tc.tile_pool(name="recpool", bufs=1))
        atpool = ctx.enter_context(tc.tile_pool(name="atpool", bufs=1))
        a8pool = ctx.enter_context(tc.tile_pool(name="a8pool", bufs=2))
        wostg = ctx.enter_context(tc.tile_pool(name="wostg", bufs=3))
        ps8 = ctx.enter_context(tc.tile_pool(name="ps8", bufs=8, space="PSUM"))

        # resident weights
        w8t = wres.tile([128, NDC * WROW], F8)
        wrt = wres.tile([128, HPC * 2 * D], F8)

        # constants
        cosT = const.tile([128, S], BF16)
        sinT = const.tile([128, S], BF16)
        dmask = const.tile([128, 4 * SQ], BF16)
        ones_t = const.tile([128, 128], F32R)
        ident = const.tile([128, 128], F32R)

        def load_consts():
            nc.sync.dma_start(cosT[0:64, :], cosq.ap())
            nc.sync.dma_start(cosT[64:128, :], cosq.ap())
            nc.sync.dma_start(sinT[0:64, :], sinq.ap())
            nc.sync.dma_start(sinT[64:128, :], sinq.ap())
            nc.sync.dma_start(dmask[:], diagm.ap())
            nc.sync.dma_start(ones_t[:], ones4.ap())

        def dma_w8(g, h0=0.0, h1=1.0):
            a = int((g + h0) * GRP * 128)
            b = int((g + h1) * GRP * 128)
            nc.sync.dma_start(
                w8t[:, a * WROW // 128 : b * WROW // 128].rearrange(
                    "p (c x) -> p c x", x=WROW
                ),
                w8.ap()[a:b, :].rearrange("(c p) x -> p c x", p=128),
            )

        # persistent q0..q3,k slabs (transposed [dim, seq]) + transposed V
        qkv = [
            [slabs.tile([128, SQ], F32R, name=f"qkv{s}_{i}") for i in range(5)]
            for s in range(NSQ)
        ]
        vt_s = [slabs.tile([128, SQ], F32R, name=f"vt{s}") for s in range(NSQ)]

        # ---- fused QKV projection: fp8 DoubleRow, PSUM-resident ----
        def proj(s, with_w=False, tail_hook=None):
            ps = [
                ps8.tile([128, SQ], F32, tag="ps", name=f"proj{s}_{nt}")
                for nt in range(NSLAB)
            ]
            for g in range(NG):
                # the very first group streams w/x in quarters so the PE
                # starts as soon as the first 2 chunks land
                halves = (
                    (0.0, 0.25, 0.5, 0.75, 1.0)
                    if (with_w and g == 0)
                    else (0.0, 1.0)
                )
                xg = xpool.tile([128, GRP * 2 * SQ], F8, tag="x")
                for hh in range(len(halves) - 1):
                    h0, h1 = halves[hh], halves[hh + 1]
                    c0, c1 = int(h0 * GRP), int(h1 * GRP)
                    if with_w:
                        dma_w8(g, h0, h1)
                    nc.sync.dma_start(
                        xg[:, c0 * 2 * SQ : c1 * 2 * SQ].rearrange(
                            "p (c n) -> p c n", n=2 * SQ
                        ),
                        x8.ap()[
                            s * D + (g * GRP + c0) * 128 : s * D
                            + (g * GRP + c1) * 128,
                            :,
                        ].rearrange("(c p) x -> p c x", p=128),
                    )
                    wv_g = w8t[:, g * GRP * WROW : (g + 1) * GRP * WROW]
                    w_ch = wv_g.rearrange("p (c x) -> p c x", x=WROW)
                    w_sl = wv_g.rearrange(
                        "p (c t x) -> p c t x", t=2, x=NSLAB * 128
                    )
                    x_pr = xg[:].rearrange("p (c n) -> p c n", n=2 * SQ)
                    x_sl = xg[:].rearrange("p (c t n) -> p c t n", t=2, n=SQ)
                    for nt in range(NSLAB):
                        co = nt * 128
                        for k in range(c0 // 2, c1 // 2):
                            # hi@hi over chunk pair (2k, 2k+1)
                            nc.tensor.matmul(
                                ps[nt][:],
                                w_ch[:, 2 * k : 2 * k + 2, co : co + 128],
                                x_pr[:, 2 * k : 2 * k + 2, SQ : 2 * SQ],
                                start=(g == 0 and k == 0),
                                stop=False,
                                perf_mode=DR,
                            )
                        last = g == NG - 1 and h1 == 1.0
                        for dd in range(c0, c1):
                            # cross terms: (w_hi, w_lo) @ (x_lo, x_hi)
                            nc.tensor.matmul(
                                ps[nt][:],
                                w_sl[:, dd, :, co : co + 128],
                                x_sl[:, dd],
                                start=False,
                                stop=(last and dd == GRP - 1),
                                perf_mode=DR,
                            )
            # V first (its transpose is on the PE critical path), then the
            # next attention chunk's pre-ramp, then the remaining copies
            # spread over ACT/DVE so banks release in parallel and no
            # cross-engine ordering inversions appear.
            vs = vtmp.tile([128, SQ], F32R, tag="v")
            nc.scalar.copy(vs[:], ps[5][:])
            if tail_hook is not None:
                tail_hook()
            for tt in range(4):
                tp = ps8.tile([128, 128], F32R, tag="ps", name=f"vtp{s}_{tt}")
                nc.tensor.transpose(
                    tp[:], vs[:, tt * 128 : (tt + 1) * 128], ident[:]
                )
                nc.scalar.copy(vt_s[s][:, tt * 128 : (tt + 1) * 128], tp[:])
            nc.scalar.copy(qkv[s][0][:], ps[0][:])
            nc.scalar.copy(qkv[s][1][:], ps[1][:])
            nc.vector.tensor_copy(qkv[s][2][:], ps[2][:])
            nc.vector.tensor_copy(qkv[s][3][:], ps[3][:])
            nc.scalar.copy(qkv[s][4][:], ps[4][:])

        def rope(s):
            cs_lo = cosT[0:64, s * SQ : (s + 1) * SQ]
            cs_hi = cosT[64:128, s * SQ : (s + 1) * SQ]
            sn_lo = sinT[0:64, s * SQ : (s + 1) * SQ]
            sn_hi = sinT[64:128, s * SQ : (s + 1) * SQ]
            for nt in (4, 0, 1, 2, 3):
                dve = nt in (4, 0, 1)
                eng = nc.vector if dve else nc.gpsimd
                pool = tmppool if dve else tmppoolp
                tl = qkv[s][nt]
                lo = tl[0:64, :]
                hi = tl[64:128, :]
                m1 = pool.tile([64, SQ], F32, tag="t")
                m2 = pool.tile([64, SQ], F32, tag="t")
                m3 = pool.tile([64, SQ], F32, tag="t")
                m4 = pool.tile([64, SQ], F32, tag="t")
                eng.tensor_mul(m1[:], lo, cs_lo)
                eng.tensor_mul(m2[:], hi, sn_hi)
                eng.tensor_mul(m3[:], lo, sn_lo)
                eng.tensor_mul(m4[:], hi, cs_hi)
                eng.tensor_sub(hi, m1[:], m2[:])   # rotated low half
                eng.tensor_add(lo, m3[:], m4[:])   # rotated high half

        def ktile(t):
            return qkv[t // 4][4][:, (t % 4) * 128 : (t % 4) * 128 + 128]

        def vtile(t):
            return vt_s[t // 4][:, (t % 4) * 128 : (t % 4) * 128 + 128]

        attout8 = {}

        # ---- attention (flash, transposed scores, causal block skip) ----
        # software-pipelined: the scores matmul + exp for tile i+L issue
        # before the av/den matmuls of tile i, so the PE never waits on the
        # ACT exp latency.
        def att_make(c, L=2, npre=2):
            """Returns (pre, rest): pre emits the first `npre` sc/exp chains
            (callable from inside the preceding proj phase, using the spare
            PSUM banks); rest emits everything else."""
            ntiles = 4 * c + 4
            avden = {}
            state = {"a8v": None}
            pend = []
            stream = [(h, t) for h in range(HPC) for t in range(ntiles)]

            def finalize(h):
                a8v = state["a8v"]
                av, den = avden.pop(h)
                rec = recpool.tile([128, SQ], F32, tag="rec")
                nc.vector.reciprocal(rec[:], den[:])
                t_f = atpool.tile([128, SQ], F32, tag="t")
                nc.vector.tensor_mul(t_f[:], av[:], rec[:])
                nc.scalar.copy(a8v[:, h, 0, :], t_f[:])     # hi (fp8 cast)
                nc.vector.tensor_sub(a8v[:, h, 1, :], t_f[:], a8v[:, h, 0, :])

            def emit_sc(h, t):
                j = t - 4 * c
                # diagonal tiles: queries below the causal frontier are all
                # masked; shrink the moving free dim (kept >= 256 for fp32r)
                qo = 0 if j < 1 else (128 if j == 1 else 256)
                fr = SQ - qo
                sc = ps8.tile([128, fr], F32, tag="ps", name=f"sc{h}_{c}_{t}")
                nc.tensor.matmul(
                    sc[:], ktile(t), qkv[c][h][:, qo:SQ], start=True, stop=True
                )
                if j >= 0:
                    nc.vector.tensor_add(
                        sc[:], sc[:], dmask[:, j * SQ + qo : (j + 1) * SQ]
                    )
                pt = ptpool.tile([128, fr], F32R, tag="pt")
                nc.scalar.activation(pt[:], sc[:], Exp)
                pend.append((h, t, qo, pt))

            def pre():
                for h, t in stream[:npre]:
                    emit_sc(h, t)

            def rest():
                a8 = a8pool.tile(
                    [128, HPC * 2 * SQ], F8, tag="a8", name=f"a8_{c}"
                )
                attout8[c] = a8
                state["a8v"] = a8[:].rearrange("p (h t n) -> p h t n", h=HPC, t=2)
                for h, t in stream[npre:]:
                    emit_sc(h, t)
                    if len(pend) > L:
                        emit_avden(c, ntiles, avden, pend.pop(0), finalize)
                while pend:
                    emit_avden(c, ntiles, avden, pend.pop(0), finalize)

            return pre, rest

        def emit_avden(c, ntiles, avden, item, finalize):
            h, t, qo, pt = item
            if t == 0:
                avden[h] = (
                    ps8.tile([128, SQ], F32, tag="ps", name=f"av{h}_{c}"),
                    ps8.tile([128, SQ], F32, tag="ps", name=f"den{h}_{c}"),
                )
            av, den = avden[h]
            last = t == ntiles - 1
            nc.tensor.matmul(
                av[:, qo:SQ], vtile(t), pt[:], start=(t == 0), stop=last
            )
            nc.tensor.matmul(
                den[:, qo:SQ], ones_t[:], pt[:], start=(t == 0), stop=last
            )
            if last:
                finalize(h)

        # ---- wo projection: fp8 DoubleRow, bf16 staged output ----
        def wo(c, last=False):
            a8 = attout8.pop(c)
            a8v = a8[:].rearrange("p (h t n) -> p h t n", h=HPC, t=2)
            wrv = wrt[:].rearrange("p (h t n) -> p h t n", h=HPC, t=2)
            for m in range(4):
                mo = m * 128
                # final row of the kernel drains per-tile so the last DMA is
                # small (shorter end-of-kernel tail)
                fine = last and m == 3
                for jq in range(2):
                    st = wostg.tile([128, 4 * SQ], BF16, tag="st")
                    for jj in range(4):
                        j = jq * 4 + jj
                        po = ps8.tile(
                            [128, SQ], F32, tag="ps", name=f"po{c}_{m}_{j}"
                        )
                        for p in range(HPC // 2):
                            # hi@hi over head pair (2p, 2p+1)
                            nc.tensor.matmul(
                                po[:],
                                a8v[:, 2 * p : 2 * p + 2, 0, mo : mo + 128],
                                wrv[:, 2 * p : 2 * p + 2, 1, j * SQ : (j + 1) * SQ],
                                start=(p == 0),
                                stop=False,
                                perf_mode=DR,
                            )
                        for h in range(HPC):
                            # cross: (a_hi, a_lo) @ (wo_lo, wo_hi)
                            nc.tensor.matmul(
                                po[:],
                                a8v[:, h, :, mo : mo + 128],
                                wrv[:, h, :, j * SQ : (j + 1) * SQ],
                                start=False,
                                stop=(h == HPC - 1),
                                perf_mode=DR,
                            )
                        # GPSIMD cannot read PSUM; alternate ACT/DVE
                        eng = (nc.scalar.copy, nc.vector.tensor_copy)[
                            (m * 8 + j) % 2
                        ]
                        eng(st[:, jj * SQ : (jj + 1) * SQ], po[:])
                        if fine:
                            nc.sync.dma_start(
                                out.ap()[
                                    (4 * c + m) * 128 : (4 * c + m + 1) * 128,
                                    j * SQ : (j + 1) * SQ,
                                ],
                                st[:, jj * SQ : (jj + 1) * SQ],
                            )
                    if not fine:
                        nc.sync.dma_start(
                            out.ap()[
                                (4 * c + m) * 128 : (4 * c + m + 1) * 128,
                                jq * 4 * SQ : (jq + 1) * 4 * SQ,
                            ],
                            st[:],
                        )

        # ---- schedule ----
        # ident MUST be emitted before proj(0): its V-transposes read it,
        # and a read emitted before the write would consume garbage
        nc.sync.dma_start(ident[:], identd.ap())
        proj(0, with_w=True)
        load_consts()
        rope(0)
        a0_pre, a0_rest = att_make(0)
        proj(1, tail_hook=a0_pre)
        a0_rest()
        rope(1)
        a1_pre, a1_rest = att_make(1)
        proj(2, tail_hook=a1_pre)
        nc.sync.dma_start(wrt[:], wor8.ap())
        a1_rest()
        wo(0)
        rope(2)
        a2_pre, a2_rest = att_make(2)
        proj(3, tail_hook=a2_pre)
        a2_rest()
        wo(1)
        rope(3)
        a3_pre, a3_rest = att_make(3)
        a3_pre()
        a3_rest()
        wo(2)
        wo(3, last=True)


def _host_prep(x, wq, wk, wv, wo, freqs_cos, freqs_sin):
    """Build the 8 per-core input maps (fp8 hi/lo splits, rescaled)."""
    perm = np.concatenate([np.arange(0, HD, 2), np.arange(1, HD, 2)])
    xt = np.ascontiguousarray(x.reshape(S, D).T)
    xhi = xt.astype(E4)
    xlo = (xt - xhi.astype(np.float32)).astype(E4)
    # s-chunk-major, (lo, hi) slots: [NSQ, D, 2, SQ] -> [NSQ * D, 2 * SQ]
    x8 = np.ascontiguousarray(
        np.stack(
            [
                xlo.reshape(D, NSQ, SQ).transpose(1, 0, 2),
                xhi.reshape(D, NSQ, SQ).transpose(1, 0, 2),
            ],
            axis=2,
        ).reshape(NSQ * D, 2 * SQ)
    )
    cosq = np.ascontiguousarray((freqs_cos.T.astype(np.float32) * BETA).astype(ml_dtypes.bfloat16))
    sinq = np.ascontiguousarray((freqs_sin.T.astype(np.float32) * BETA).astype(ml_dtypes.bfloat16))
    kk = np.arange(128)[:, None]
    qq = np.arange(SQ)[None, :]
    diagm = np.concatenate(
        [
            np.where(128 * j + kk <= qq, 0.0, -1e9).astype(ml_dtypes.bfloat16)
            for j in range(4)
        ],
        axis=1,
    )
    ones4 = np.full((128, 128), CONES, np.float32)
    ident = np.eye(128, dtype=np.float32)

    in_maps = []
    for c in range(NCORES):
        wq_c = (
            wq[:, (HPC * c) * HD : (HPC * c + HPC) * HD]
            .reshape(D, HPC, HD)[:, :, perm]
            .reshape(D, HPC * HD)
        )
        wk_c = wk[:, c * HD : (c + 1) * HD][:, perm]
        wv_c = wv[:, c * HD : (c + 1) * HD]
        wcat = np.concatenate([wq_c, wk_c, wv_c], axis=1) * WSC  # [D, 768]
        whi = wcat.astype(E4)
        wlo = (wcat - whi.astype(np.float32)).astype(E4)
        w8 = np.ascontiguousarray(
            np.stack([whi, wlo], axis=1).reshape(D, 2 * NSLAB * 128)
        )  # (hi, lo)
        wo_c = (
            wo[(HPC * c) * HD : (HPC * c + HPC) * HD, :].reshape(HPC, 128, D)
            * WSC
        )
        wo_hd = wo_c.transpose(1, 0, 2)  # [128 hd, HPC, D]
        whi_o = wo_hd.astype(E4)
        wlo_o = (wo_hd - whi_o.astype(np.float32)).astype(E4)
        wor8 = np.ascontiguousarray(
            np.stack([wlo_o, whi_o], axis=2).reshape(128, HPC * 2 * D)
        )  # (lo, hi)
        in_maps.append(
            {
                "w8": w8,
                "x8": x8,
                "wor8": wor8,
                "cosq": cosq,
                "sinq": sinq,
                "diagm": diagm,
                "ones4": ones4,
                "identd": ident,
            }
        )
    return in_maps


def _numpy_fallback(x, wq, wk, wv, wo, freqs_cos, freqs_sin, mask):
    """Exact reference math in numpy (used only for non-causal masks)."""
    bsz = x.shape[0]
    n_rep = H // H_KV
    xq = (x.reshape(-1, D) @ wq).reshape(bsz, S, H, HD)
    xk = (x.reshape(-1, D) @ wk).reshape(bsz, S, H_KV, HD)
    xv = (x.reshape(-1, D) @ wv).reshape(bsz, S, H_KV, HD)

    def rope(t):
        t0, t1 = t[..., 0::2], t[..., 1::2]
        c = freqs_cos[None, :, None, :]
        s = freqs_sin[None, :, None, :]
        o0 = t0 * c - t1 * s
        o1 = t0 * s + t1 * c
        return np.stack([o0, o1], axis=-1).reshape(t.shape)

    xq, xk = rope(xq), rope(xk)
    keys = np.repeat(xk, n_rep, axis=2)
    values = np.repeat(xv, n_rep, axis=2)
    scores = np.einsum("bqhd,bkhd->bhqk", xq, keys) / math.sqrt(HD)
    scores = scores + mask[:, :, -S:, -S:]
    scores = scores - scores.max(axis=-1, keepdims=True)
    e = np.exp(scores)
    attn = e / e.sum(axis=-1, keepdims=True)
    o = np.einsum("bhqk,bkhd->bqhd", attn, values).reshape(bsz, S, H * HD)
    return (o @ wo).astype(np.float32)


def kernel(**inputs):
    x = np.asarray(inputs["x"], dtype=np.float32)
    wq = np.asarray(inputs["wq"], dtype=np.float32)
    wk = np.asarray(inputs["wk"], dtype=np.float32)
    wv = np.asarray(inputs["wv"], dtype=np.float32)
    wo = np.asarray(inputs["wo"], dtype=np.float32)
    fc = np.asarray(inputs["freqs_cos"], dtype=np.float32)
    fs = np.asarray(inputs["freqs_sin"], dtype=np.float32)
    mask = np.asarray(inputs["mask"], dtype=np.float32)

    causal = np.triu(np.full((S, S), -1e9, dtype=np.float32), k=1)[None, None]
    if x.shape != (1, S, D) or not np.array_equal(mask, causal):
        return _numpy_fallback(x, wq, wk, wv, wo, fc, fs, mask)

    if "nc" not in _NC_CACHE:
        _NC_CACHE["nc"] = _build_nc()
    nc = _NC_CACHE["nc"]
    in_maps = _host_prep(x[0], wq, wk, wv, wo, fc, fs)
    _log("launching on 8 cores (compile on first call + transfers)")
    res = run_bass_kernel_spmd(nc, in_maps, core_ids=list(range(NCORES)))
    _log("run complete")
    full = np.zeros((S, D), np.float32)
    for r in res.results:
        full += r["out"].astype(np.float32)
    full /= OUT_DIV
    return full.reshape(1, S, D)


# revision 4
# speedup vs baseline: 1.0304x; 1.0304x over previous
"""Trainium2 Bass kernel for GQA attention (B=1, S=2048, D=4096, H=32, H_KV=8, HD=128).

Sharding: tensor-parallel over heads, 8 cores; core c owns Q heads 4c..4c+3 and
KV head c.  Each core computes a partial [S, D] output (wo row-shard); the host
sums the partials.

Per-core kernel v2 (fp8-DoubleRow compensated projections):
  - QKV projection and the wo output projection run as fp8e4m3 DoubleRow
    matmuls (cost-model 0.5 cycles/row, 256-deep contraction per
    instruction).  Full fp32-level accuracy is recovered with a hi/lo
    split: w = w_hi + w_lo, x = x_hi + x_lo (host-side, power-of-2
    rescaled so both splits stay in fp8's normal range), computing
    w_hi@x_hi (chunk-paired) + (w_hi@x_lo + w_lo@x_hi) (slot-paired in a
    single DoubleRow op).  The dropped w_lo@x_lo term is ~1e-3 relative.
  - All scale factors fold into free spots: the RoPE cos/sin tables carry
    the score scale, the softmax-denominator ones-matrix carries the V
    rescale, and the host divides the summed output by a single constant.
  - QKV accumulates entirely in PSUM (6 banks), no SBUF folding; weights
    and x stream once (fp8, half traffic).
  - Attention: flash-style transposed scores (fp32r), causal block skip,
    softmax denominator via a 4.0-matrix matmul (fp32r); normalization and
    the attout hi/lo fp8 split happen on DVE/ACT.
  - Output staged to bf16 in SBUF, DMA'd out, host sums in fp32.
  - PE phase order: p0 p1 a0 p2 a1 w0 p3 a2 w1 a3 w2 w3 so RoPE (DVE),
    exp (ACT), attout finalization and output DMA all hide under PE work.
"""

import math
import os
import sys
import time

import numpy as np
import ml_dtypes

E4 = ml_dtypes.float8_e4m3fn


def _log(msg):
    if os.environ.get("KERNEL_QUIET"):
        return
    print(f"[kernel {time.strftime('%H:%M:%S')}] {msg}", file=sys.stderr, flush=True)


import concourse.bass as bass
import concourse.tile as tile
from concourse import bacc, mybir
from concourse.bass_utils import run_bass_kernel_spmd

S, D = 2048, 4096
H, H_KV, HD = 32, 8, 128
NCORES = 8
HPC = H // NCORES            # 4 Q heads per core
NSLAB = 6                    # q0..q3, k, v slabs of 128 cols
SQ = 512
NSQ = S // SQ                # 4
NKT = S // 128               # 16 key tiles
NDC = D // 128               # 32 contraction chunks
GRP = 8                      # chunks per DMA group
NG = NDC // GRP              # 4 groups
F32 = mybir.dt.float32
F32R = mybir.dt.float32r
F8 = mybir.dt.float8e4
BF16 = mybir.dt.bfloat16
DR = mybir.MatmulPerfMode.DoubleRow
Exp = mybir.ActivationFunctionType.Exp

WSC = 64.0                       # weight rescale for fp8 range
BETA = 1.0 / (WSC * 128.0 ** 0.25)   # rope cos/sin scale (per q and k)
CONES = 4.0                      # denominator matrix value -> attout = 16x true
OUT_DIV = (WSC / CONES) * WSC    # host divides summed output by this (1024)

_NC_CACHE = {}


def _build_nc():
    nc = bacc.Bacc(
        "TRN2", target_bir_lowering=False, debug=False, enable_asserts=False
    )
    w8 = nc.dram_tensor("w8", [D, 2 * NSLAB * 128], F8, kind="ExternalInput")
    # s-chunk-major: [NSQ * D, (lo, hi) * SQ] so each (g, s) load is 3-dim
    x8 = nc.dram_tensor("x8", [NSQ * D, 2 * SQ], F8, kind="ExternalInput")
    wor8 = nc.dram_tensor("wor8", [128, HPC * 2 * D], F8, kind="ExternalInput")
    cosq = nc.dram_tensor("cosq", [64, S], BF16, kind="ExternalInput")
    sinq = nc.dram_tensor("sinq", [64, S], BF16, kind="ExternalInput")
    diagm = nc.dram_tensor("diagm", [128, 256], BF16, kind="ExternalInput")
    ones4 = nc.dram_tensor("ones4", [128, 128], F32R, kind="ExternalInput")
    identd = nc.dram_tensor("identd", [128, 128], F32R, kind="ExternalInput")
    out = nc.dram_tensor("out", [S, D], BF16, kind="ExternalOutput")

    _log("emitting IR")
    with tile.TileContext(nc) as tc:
        _emit(tc, w8, x8, wor8, cosq, sinq, diagm, ones4, identd, out)
    _log("bacc compile")
    nc.compile()
    _log("bass module ready")
    return nc


def _emit(tc, w8, x8, wor8, cosq, sinq, diagm, ones4, identd, out):
    from contextlib import ExitStack

    nc = tc.nc
    WROW = 2 * NSLAB * 128       # 1536 fp8 cols per d-chunk of w8
    with ExitStack() as ctx:
        const = ctx.enter_context(tc.tile_pool(name="const", bufs=1))
        wres = ctx.enter_context(tc.tile_pool(name="wres", bufs=1))
        slabs = ctx.enter_context(tc.tile_pool(name="slabs", bufs=1))
        xpool = ctx.enter_context(tc.tile_pool(name="xpool", bufs=3))
        vtmp = ctx.enter_context(tc.tile_pool(name="vtmp", bufs=1))
        ptpool = ctx.enter_context(tc.tile_pool(name="ptpool", bufs=5))
        # separate rope scratch rings per engine: a shared ring would make
        # the Pool rope ops wait on DVE's tile releases (serializing them)
        tmppool = ctx.enter_context(tc.tile_pool(name="tmppool", bufs=4))
        tmppoolp = ctx.enter_context(tc.tile_pool(name="tmppoolp", bufs=4))
        recpool = ctx.enter_context(tc.tile_pool(name="recpool", bufs=1))
        atpool = ctx.enter_context(tc.tile_pool(name="atpool", bufs=1))
        a8pool = ctx.enter_context(tc.tile_pool(name="a8pool", bufs=2))
        wostg = ctx.enter_context(tc.tile_pool(name="wostg", bufs=3))
        ps8 = ctx.enter_context(tc.tile_pool(name="ps8", bufs=8, space="PSUM"))

        # resident weights
        w8t = wres.tile([128, NDC * WROW], F8)
        wrt = wres.tile([128, HPC * 2 * D], F8)

        # constants
        cosT = const.tile([128, S], BF16)
        sinT = const.tile([128, S], BF16)
        dmask = const.tile([128, 256], BF16)
        ones_t = const.tile([128, 128], F32R)
        ident = const.tile([128, 128], F32R)

        def load_consts():
            nc.sync.dma_start(cosT[0:64, :], cosq.ap())
            nc.sync.dma_start(cosT[64:128, :], cosq.ap())
            nc.sync.dma_start(sinT[0:64, :], sinq.ap())
            nc.sync.dma_start(sinT[64:128, :], sinq.ap())
            nc.sync.dma_start(dmask[:], diagm.ap())
            nc.sync.dma_start(ones_t[:], ones4.ap())

        def dma_w8(g, h0=0.0, h1=1.0):
            a = int((g + h0) * GRP * 128)
            b = int((g + h1) * GRP * 128)
            nc.sync.dma_start(
                w8t[:, a * WROW // 128 : b * WROW // 128].rearrange(
                    "p (c x) -> p c x", x=WROW
                ),
                w8.ap()[a:b, :].rearrange("(c p) x -> p c x", p=128),
            )

        # persistent q0..q3,k slabs (transposed [dim, seq]) + transposed V
        qkv = [
            [slabs.tile([128, SQ], F32R, name=f"qkv{s}_{i}") for i in range(5)]
            for s in range(NSQ)
        ]
        vt_s = [slabs.tile([128, SQ], F32R, name=f"vt{s}") for s in range(NSQ)]

        # ---- fused QKV projection: fp8 DoubleRow, PSUM-resident ----
        def proj(s, with_w=False, tail_hook=None):
            ps = [
                ps8.tile([128, SQ], F32, tag="ps", name=f"proj{s}_{nt}")
                for nt in range(NSLAB)
            ]
            started = [False] * NSLAB
            HG = GRP // 2        # chunks per x half-group tile
            for g in range(NG):
                for hf in range(2):
                    # the very first half-group streams w/x in single chunks
                    # so the PE starts as soon as the first chunk lands
                    first_hg = with_w and g == 0 and hf == 0
                    subs = (0.0, 0.25, 0.5, 1.0) if first_hg else (0.0, 1.0)
                    xg = xpool.tile([128, HG * 2 * SQ], F8, tag="x")
                    base = g * GRP + hf * HG     # absolute first chunk
                    for hh in range(len(subs) - 1):
                        c0 = int(subs[hh] * HG)
                        c1 = int(subs[hh + 1] * HG)
                        if with_w:
                            dma_w8(g, (hf * HG + c0) / GRP, (hf * HG + c1) / GRP)
                        nc.sync.dma_start(
                            xg[:, c0 * 2 * SQ : c1 * 2 * SQ].rearrange(
                                "p (c n) -> p c n", n=2 * SQ
                            ),
                            x8.ap()[
                                s * D + (base + c0) * 128 : s * D
                                + (base + c1) * 128,
                                :,
                            ].rearrange("(c p) x -> p c x", p=128),
                        )
                        wv_g = w8t[:, base * WROW : (base + HG) * WROW]
                        w_ch = wv_g.rearrange("p (c x) -> p c x", x=WROW)
                        w_sl = wv_g.rearrange(
                            "p (c t x) -> p c t x", t=2, x=NSLAB * 128
                        )
                        x_pr = xg[:].rearrange("p (c n) -> p c n", n=2 * SQ)
                        x_sl = xg[:].rearrange(
                            "p (c t n) -> p c t n", t=2, n=SQ
                        )
                        for nt in range(NSLAB):
                            co = nt * 128
                            # emit cross terms first (each needs only one
                            # chunk, letting the first matmuls start before
                            # the pair's second chunk lands)
                            last = g == NG - 1 and hf == 1 and c1 == HG
                            for dd in range(c0, c1):
                                # cross terms: (w_hi, w_lo) @ (x_lo, x_hi)
                                nc.tensor.matmul(
                                    ps[nt][:],
                                    w_sl[:, dd, :, co : co + 128],
                                    x_sl[:, dd],
                                    start=not started[nt],
                                    stop=False,
                                    perf_mode=DR,
                                )
                                started[nt] = True
                            for k in range(c0 // 2, (c1 + 1) // 2):
                                # hi@hi over chunk pair (2k, 2k+1) of the
                                # half-group; emitted once both chunks of
                                # the pair are covered
                                if 2 * k + 2 > c1:
                                    continue
                                nc.tensor.matmul(
                                    ps[nt][:],
                                    w_ch[:, 2 * k : 2 * k + 2, co : co + 128],
                                    x_pr[:, 2 * k : 2 * k + 2, SQ : 2 * SQ],
                                    start=False,
                                    stop=(last and 2 * k + 2 == HG),
                                    perf_mode=DR,
                                )
            # V first (its transpose is on the PE critical path), then the
            # next attention chunk's pre-ramp, then the remaining copies
            # spread over ACT/DVE so banks release in parallel and no
            # cross-engine ordering inversions appear.
            vs = vtmp.tile([128, SQ], F32R, tag="v")
            nc.scalar.copy(vs[:], ps[5][:])
            if tail_hook is not None:
                tail_hook()
            for tt in range(4):
                tp = ps8.tile([128, 128], F32R, tag="ps", name=f"vtp{s}_{tt}")
                nc.tensor.transpose(
                    tp[:], vs[:, tt * 128 : (tt + 1) * 128], ident[:]
                )
                nc.scalar.copy(vt_s[s][:, tt * 128 : (tt + 1) * 128], tp[:])
            nc.scalar.copy(qkv[s][0][:], ps[0][:])
            nc.scalar.copy(qkv[s][1][:], ps[1][:])
            nc.vector.tensor_copy(qkv[s][2][:], ps[2][:])
            nc.vector.tensor_copy(qkv[s][3][:], ps[3][:])
            nc.scalar.copy(qkv[s][4][:], ps[4][:])

        def rope(s):
            cs_lo = cosT[0:64, s * SQ : (s + 1) * SQ]
            cs_hi = cosT[64:128, s * SQ : (s + 1) * SQ]
            sn_lo = sinT[0:64, s * SQ : (s + 1) * SQ]
            sn_hi = sinT[64:128, s * SQ : (s + 1) * SQ]
            for nt in (4, 0, 1, 2, 3):
                dve = nt in (4, 0)
                eng = nc.vector if dve else nc.gpsimd
                pool = tmppool if dve else tmppoolp
                tl = qkv[s][nt]
                lo = tl[0:64, :]
                hi = tl[64:128, :]
                m1 = pool.tile([64, SQ], F32, tag="t")
                m2 = pool.tile([64, SQ], F32, tag="t")
                m3 = pool.tile([64, SQ], F32, tag="t")
                m4 = pool.tile([64, SQ], F32, tag="t")
                eng.tensor_mul(m1[:], lo, cs_lo)
                eng.tensor_mul(m2[:], hi, sn_hi)
                eng.tensor_mul(m3[:], lo, sn_lo)
                eng.tensor_mul(m4[:], hi, cs_hi)
                eng.tensor_sub(hi, m1[:], m2[:])   # rotated low half
                eng.tensor_add(lo, m3[:], m4[:])   # rotated high half

        def ktile(t):
            return qkv[t // 4][4][:, (t % 4) * 128 : (t % 4) * 128 + 128]

        def vtile(t):
            return vt_s[t // 4][:, (t % 4) * 128 : (t % 4) * 128 + 128]

        attout8 = {}

        # ---- attention (flash, transposed scores, causal block skip) ----
        # software-pipelined: the scores matmul + exp for tile i+L issue
        # before the av/den matmuls of tile i, so the PE never waits on the
        # ACT exp latency.
        def att_make(c, L=3, npre=6):
            """Returns (pre, rest): pre emits the first `npre` sc/exp chains
            (callable from inside the preceding proj phase, using the spare
            PSUM banks); rest emits everything else."""
            ntiles = 4 * c + 4
            avden = {}
            state = {"a8v": None}
            pend = []
            stream = [(h, t) for h in range(HPC) for t in range(ntiles)]

            def finalize(h):
                a8v = state["a8v"]
                av, den = avden.pop(h)
                rec = recpool.tile([128, SQ], F32, tag="rec")
                nc.vector.reciprocal(rec[:], den[:])
                t_f = atpool.tile([128, SQ], F32, tag="t")
                nc.vector.tensor_mul(t_f[:], av[:], rec[:])
                nc.vector.tensor_copy(a8v[:, h, 0, :], t_f[:])  # hi (fp8 cast)
                nc.vector.tensor_sub(a8v[:, h, 1, :], t_f[:], a8v[:, h, 0, :])

            def emit_sc(h, t):
                j = t - 4 * c
                # diagonal tiles: queries below the causal frontier are all
                # masked; shrink the moving free dim (kept >= 256 for fp32r)
                qo = 0 if j < 1 else (128 if j == 1 else 256)
                fr = SQ - qo
                sc = ps8.tile([128, fr], F32, tag="ps", name=f"sc{h}_{c}_{t}")
                nc.tensor.matmul(
                    sc[:], ktile(t), qkv[c][h][:, qo:SQ], start=True, stop=True
                )
                if j >= 0:
                    # only the 128-wide causal boundary block needs masking
                    # (plus one fully-masked block for j=3 whose q-slice
                    # starts below the frontier)
                    if j == 3:
                        nc.vector.tensor_add(
                            sc[:, 0:256], sc[:, 0:256], dmask[:, 0:256]
                        )
                    else:
                        nc.vector.tensor_add(
                            sc[:, 0:128], sc[:, 0:128], dmask[:, 128:256]
                        )
                pt = ptpool.tile([128, fr], F32R, tag="pt")
                nc.scalar.activation(pt[:], sc[:], Exp)
                pend.append((h, t, qo, pt))

            def pre():
                for h, t in stream[:npre]:
                    emit_sc(h, t)

            def rest():
                a8 = a8pool.tile(
                    [128, HPC * 2 * SQ], F8, tag="a8", name=f"a8_{c}"
                )
                attout8[c] = a8
                state["a8v"] = a8[:].rearrange("p (h t n) -> p h t n", h=HPC, t=2)
                for h, t in stream[npre:]:
                    emit_sc(h, t)
                    if len(pend) > L:
                        emit_avden(c, ntiles, avden, pend.pop(0), finalize)
                while pend:
                    emit_avden(c, ntiles, avden, pend.pop(0), finalize)

            return pre, rest

        def emit_avden(c, ntiles, avden, item, finalize):
            h, t, qo, pt = item
            if t == 0:
                avden[h] = (
                    ps8.tile([128, SQ], F32, tag="ps", name=f"av{h}_{c}"),
                    ps8.tile([128, SQ], F32, tag="ps", name=f"den{h}_{c}"),
                )
            av, den = avden[h]
            last = t == ntiles - 1
            nc.tensor.matmul(
                av[:, qo:SQ], vtile(t), pt[:], start=(t == 0), stop=last
            )
            nc.tensor.matmul(
                den[:, qo:SQ], ones_t[:], pt[:], start=(t == 0), stop=last
            )
            if last:
                finalize(h)

        # ---- wo projection: fp8 DoubleRow, bf16 staged output ----
        def wo(c, last=False):
            a8 = attout8.pop(c)
            a8v = a8[:].rearrange("p (h t n) -> p h t n", h=HPC, t=2)
            wrv = wrt[:].rearrange("p (h t n) -> p h t n", h=HPC, t=2)
            for m in range(4):
                mo = m * 128
                # final row of the kernel drains per-tile so the last DMA is
                # small (shorter end-of-kernel tail)
                fine = last and m == 3
                for jq in range(2):
                    st = wostg.tile([128, 4 * SQ], BF16, tag="st")
                    for jj in range(4):
                        j = jq * 4 + jj
                        po = ps8.tile(
                            [128, SQ], F32, tag="ps", name=f"po{c}_{m}_{j}"
                        )
                        for p in range(HPC // 2):
                            # hi@hi over head pair (2p, 2p+1)
                            nc.tensor.matmul(
                                po[:],
                                a8v[:, 2 * p : 2 * p + 2, 0, mo : mo + 128],
                                wrv[:, 2 * p : 2 * p + 2, 1, j * SQ : (j + 1) * SQ],
                                start=(p == 0),
                                stop=False,
                                perf_mode=DR,
                            )
                        for h in range(HPC):
                            # cross: (a_hi, a_lo) @ (wo_lo, wo_hi)
                            nc.tensor.matmul(
                                po[:],
                                a8v[:, h, :, mo : mo + 128],
                                wrv[:, h, :, j * SQ : (j + 1) * SQ],
                                start=False,
                                stop=(h == HPC - 1),
                                perf_mode=DR,
                            )
                        # GPSIMD cannot read PSUM; alternate ACT/DVE
                        eng = (nc.scalar.copy, nc.vector.tensor_copy)[
                            (m * 8 + j) % 2
                        ]
                        eng(st[:, jj * SQ : (jj + 1) * SQ], po[:])
                        if fine:
                            nc.sync.dma_start(
                                out.ap()[
                                    (4 * c + m) * 128 : (4 * c + m + 1) * 128,
                                    j * SQ : (j + 1) * SQ,
                                ],
                                st[:, jj * SQ : (jj + 1) * SQ],
                            )
                    if not fine:
                        nc.sync.dma_start(
                            out.ap()[
                                (4 * c + m) * 128 : (4 * c + m + 1) * 128,
                                jq * 4 * SQ : (jq + 1) * 4 * SQ,
                            ],
                            st[:],
                        )

        # ---- schedule ----
        # ident MUST be emitted before proj(0): its V-transposes read it,
        # and a read emitted before the write would consume garbage
        nc.sync.dma_start(ident[:], identd.ap())
        proj(0, with_w=True)
        load_consts()
        rope(0)
        a0_pre, a0_rest = att_make(0)
        proj(1, tail_hook=a0_pre)
        a0_rest()
        rope(1)
        a1_pre, a1_rest = att_make(1)
        proj(2, tail_hook=a1_pre)
        nc.sync.dma_start(wrt[:], wor8.ap())
        a1_rest()
        wo(0)
        rope(2)
        a2_pre, a2_rest = att_make(2)
        proj(3, tail_hook=a2_pre)
        a2_rest()
        wo(1)
        rope(3)
        a3_pre, a3_rest = att_make(3)
        a3_pre()
        a3_rest()
        wo(2)
        wo(3, last=True)


def _host_prep(x, wq, wk, wv, wo, freqs_cos, freqs_sin):
    """Build the 8 per-core input maps (fp8 hi/lo splits, rescaled)."""
    perm = np.concatenate([np.arange(0, HD, 2), np.arange(1, HD, 2)])
    xt = np.ascontiguousarray(x.reshape(S, D).T)
    xhi = xt.astype(E4)
    xlo = (xt - xhi.astype(np.float32)).astype(E4)
    # s-chunk-major, (lo, hi) slots: [NSQ, D, 2, SQ] -> [NSQ * D, 2 * SQ]
    x8 = np.ascontiguousarray(
        np.stack(
            [
                xlo.reshape(D, NSQ, SQ).transpose(1, 0, 2),
                xhi.reshape(D, NSQ, SQ).transpose(1, 0, 2),
            ],
            axis=2,
        ).reshape(NSQ * D, 2 * SQ)
    )
    cosq = np.ascontiguousarray((freqs_cos.T.astype(np.float32) * BETA).astype(ml_dtypes.bfloat16))
    sinq = np.ascontiguousarray((freqs_sin.T.astype(np.float32) * BETA).astype(ml_dtypes.bfloat16))
    kk = np.arange(128)[:, None]
    qq = np.arange(128)[None, :]
    tri = np.where(kk <= qq, 0.0, -1e9).astype(np.float32)
    diagm = np.concatenate(
        [np.full((128, 128), -1e9, np.float32), tri], axis=1
    ).astype(ml_dtypes.bfloat16)
    ones4 = np.full((128, 128), CONES, np.float32)
    ident = np.eye(128, dtype=np.float32)

    in_maps = []
    for c in range(NCORES):
        wq_c = (
            wq[:, (HPC * c) * HD : (HPC * c + HPC) * HD]
            .reshape(D, HPC, HD)[:, :, perm]
            .reshape(D, HPC * HD)
        )
        wk_c = wk[:, c * HD : (c + 1) * HD][:, perm]
        wv_c = wv[:, c * HD : (c + 1) * HD]
        wcat = np.concatenate([wq_c, wk_c, wv_c], axis=1) * WSC  # [D, 768]
        whi = wcat.astype(E4)
        wlo = (wcat - whi.astype(np.float32)).astype(E4)
        w8 = np.ascontiguousarray(
            np.stack([whi, wlo], axis=1).reshape(D, 2 * NSLAB * 128)
        )  # (hi, lo)
        wo_c = (
            wo[(HPC * c) * HD : (HPC * c + HPC) * HD, :].reshape(HPC, 128, D)
            * WSC
        )
        wo_hd = wo_c.transpose(1, 0, 2)  # [128 hd, HPC, D]
        whi_o = wo_hd.astype(E4)
        wlo_o = (wo_hd - whi_o.astype(np.float32)).astype(E4)
        wor8 = np.ascontiguousarray(
            np.stack([wlo_o, whi_o], axis=2).reshape(128, HPC * 2 * D)
        )  # (lo, hi)
        in_maps.append(
            {
                "w8": w8,
                "x8": x8,
                "wor8": wor8,
                "cosq": cosq,
                "sinq": sinq,
                "diagm": diagm,
                "ones4": ones4,
                "identd": ident,
            }
        )
    return in_maps


def _numpy_fallback(x, wq, wk, wv, wo, freqs_cos, freqs_sin, mask):
    """Exact reference math in numpy (used only for non-causal masks)."""
    bsz = x.shape[0]
    n_rep = H // H_KV
    xq = (x.reshape(-1, D) @ wq).reshape(bsz, S, H, HD)
    xk = (x.reshape(-1, D) @ wk).reshape(bsz, S, H_KV, HD)
    xv = (x.reshape(-1, D) @ wv).reshape(bsz, S, H_KV, HD)

    def rope(t):
        t0, t1 = t[..., 0::2], t[..., 1::2]
        c = freqs_cos[None, :, None, :]
        s = freqs_sin[None, :, None, :]
        o0 = t0 * c - t1 * s
        o1 = t0 * s + t1 * c
        return np.stack([o0, o1], axis=-1).reshape(t.shape)

    xq, xk = rope(xq), rope(xk)
    keys = np.repeat(xk, n_rep, axis=2)
    values = np.repeat(xv, n_rep, axis=2)
    scores = np.einsum("bqhd,bkhd->bhqk", xq, keys) / math.sqrt(HD)
    scores = scores + mask[:, :, -S:, -S:]
    scores = scores - scores.max(axis=-1, keepdims=True)
    e = np.exp(scores)
    attn = e / e.sum(axis=-1, keepdims=True)
    o = np.einsum("bhqk,bkhd->bqhd", attn, values).reshape(bsz, S, H * HD)
    return (o @ wo).astype(np.float32)


def kernel(**inputs):
    x = np.asarray(inputs["x"], dtype=np.float32)
    wq = np.asarray(inputs["wq"], dtype=np.float32)
    wk = np.asarray(inputs["wk"], dtype=np.float32)
    wv = np.asarray(inputs["wv"], dtype=np.float32)
    wo = np.asarray(inputs["wo"], dtype=np.float32)
    fc = np.asarray(inputs["freqs_cos"], dtype=np.float32)
    fs = np.asarray(inputs["freqs_sin"], dtype=np.float32)
    mask = np.asarray(inputs["mask"], dtype=np.float32)

    causal = np.triu(np.full((S, S), -1e9, dtype=np.float32), k=1)[None, None]
    if x.shape != (1, S, D) or not np.array_equal(mask, causal):
        return _numpy_fallback(x, wq, wk, wv, wo, fc, fs, mask)

    if "nc" not in _NC_CACHE:
        _NC_CACHE["nc"] = _build_nc()
    nc = _NC_CACHE["nc"]
    in_maps = _host_prep(x[0], wq, wk, wv, wo, fc, fs)
    _log("launching on 8 cores (compile on first call + transfers)")
    res = run_bass_kernel_spmd(nc, in_maps, core_ids=list(range(NCORES)))
    _log("run complete")
    full = np.zeros((S, D), np.float32)
    for r in res.results:
        full += r["out"].astype(np.float32)
    full /= OUT_DIV
    return full.reshape(1, S, D)


# revision 5
# speedup vs baseline: 1.0348x; 1.0043x over previous
"""Trainium2 Bass kernel for GQA attention (B=1, S=2048, D=4096, H=32, H_KV=8, HD=128).

Sharding: tensor-parallel over heads, 8 cores; core c owns Q heads 4c..4c+3 and
KV head c.  Each core computes a partial [S, D] output (wo row-shard); the host
sums the partials.

Per-core kernel v2 (fp8-DoubleRow compensated projections):
  - QKV projection and the wo output projection run as fp8e4m3 DoubleRow
    matmuls (cost-model 0.5 cycles/row, 256-deep contraction per
    instruction).  Full fp32-level accuracy is recovered with a hi/lo
    split: w = w_hi + w_lo, x = x_hi + x_lo (host-side, power-of-2
    rescaled so both splits stay in fp8's normal range), computing
    w_hi@x_hi (chunk-paired) + (w_hi@x_lo + w_lo@x_hi) (slot-paired in a
    single DoubleRow op).  The dropped w_lo@x_lo term is ~1e-3 relative.
  - All scale factors fold into free spots: the RoPE cos/sin tables carry
    the score scale, the softmax-denominator ones-matrix carries the V
    rescale, and the host divides the summed output by a single constant.
  - QKV accumulates entirely in PSUM (6 banks), no SBUF folding; weights
    and x stream once (fp8, half traffic).
  - Attention: flash-style transposed scores (fp32r), software-pipelined
    with a 3-tile scores/exp lookahead (plus a pre-ramp of 6 tiles emitted
    inside the preceding projection phase).  Causal handling skips
    above-diagonal key tiles, shrinks diagonal-tile matmuls to the live
    query range (>=256 free for fp32r full rate), and masks only the
    128-wide causal boundary block.  The softmax denominator comes from a
    4.0-matrix matmul; normalization and the attout hi/lo fp8 split run on
    DVE; RoPE is split DVE/GpSimd with per-engine scratch pools.
  - Output staged to bf16 in SBUF (ACT/DVE alternating copies, quad-batched
    DMAs; per-tile DMAs on the final row), host sums partials in fp32.
  - PE phase order: p0 p1 a0 p2 a1 w0 p3 a2 w1 a3 w2 w3 so RoPE, exp,
    attout finalization and output DMA all hide under PE work.  TimelineSim:
    319095 ns (baseline 468245 ns), PE busy 299 us (94%).
"""

import math
import os
import sys
import time

import numpy as np
import ml_dtypes

E4 = ml_dtypes.float8_e4m3fn


def _log(msg):
    if os.environ.get("KERNEL_QUIET"):
        return
    print(f"[kernel {time.strftime('%H:%M:%S')}] {msg}", file=sys.stderr, flush=True)


import concourse.bass as bass
import concourse.tile as tile
from concourse import bacc, mybir
from concourse.bass_utils import run_bass_kernel_spmd

S, D = 2048, 4096
H, H_KV, HD = 32, 8, 128
NCORES = 8
HPC = H // NCORES            # 4 Q heads per core
NSLAB = 6                    # q0..q3, k, v slabs of 128 cols
SQ = 512
NSQ = S // SQ                # 4
NKT = S // 128               # 16 key tiles
NDC = D // 128               # 32 contraction chunks
GRP = 8                      # chunks per DMA group
NG = NDC // GRP              # 4 groups
F32 = mybir.dt.float32
F32R = mybir.dt.float32r
F8 = mybir.dt.float8e4
BF16 = mybir.dt.bfloat16
DR = mybir.MatmulPerfMode.DoubleRow
Exp = mybir.ActivationFunctionType.Exp

WSC = 64.0                       # weight rescale for fp8 range
BETA = 1.0 / (WSC * 128.0 ** 0.25)   # rope cos/sin scale (per q and k)
CONES = 4.0                      # denominator matrix value -> attout = 16x true
OUT_DIV = (WSC / CONES) * WSC    # host divides summed output by this (1024)

_NC_CACHE = {}


def _build_nc():
    nc = bacc.Bacc(
        "TRN2", target_bir_lowering=False, debug=False, enable_asserts=False
    )
    w8 = nc.dram_tensor("w8", [D, 2 * NSLAB * 128], F8, kind="ExternalInput")
    # s-chunk-major: [NSQ * D, (lo, hi) * SQ] so each (g, s) load is 3-dim
    x8 = nc.dram_tensor("x8", [NSQ * D, 2 * SQ], F8, kind="ExternalInput")
    wor8 = nc.dram_tensor("wor8", [128, HPC * 2 * D], F8, kind="ExternalInput")
    cosq = nc.dram_tensor("cosq", [64, S], BF16, kind="ExternalInput")
    sinq = nc.dram_tensor("sinq", [64, S], BF16, kind="ExternalInput")
    diagm = nc.dram_tensor("diagm", [128, 256], BF16, kind="ExternalInput")
    ones4 = nc.dram_tensor("ones4", [128, 128], F32R, kind="ExternalInput")
    identd = nc.dram_tensor("identd", [128, 128], F32R, kind="ExternalInput")
    out = nc.dram_tensor("out", [S, D], BF16, kind="ExternalOutput")

    _log("emitting IR")
    with tile.TileContext(nc) as tc:
        _emit(tc, w8, x8, wor8, cosq, sinq, diagm, ones4, identd, out)
    _log("bacc compile")
    nc.compile()
    _log("bass module ready")
    return nc


def _emit(tc, w8, x8, wor8, cosq, sinq, diagm, ones4, identd, out):
    from contextlib import ExitStack

    nc = tc.nc
    WROW = 2 * NSLAB * 128       # 1536 fp8 cols per d-chunk of w8
    with ExitStack() as ctx:
        const = ctx.enter_context(tc.tile_pool(name="const", bufs=1))
        wres = ctx.enter_context(tc.tile_pool(name="wres", bufs=1))
        slabs = ctx.enter_context(tc.tile_pool(name="slabs", bufs=1))
        xpool = ctx.enter_context(tc.tile_pool(name="xpool", bufs=3))
        vtmp = ctx.enter_context(tc.tile_pool(name="vtmp", bufs=1))
        ptpool = ctx.enter_context(tc.tile_pool(name="ptpool", bufs=5))
        # separate rope scratch rings per engine: a shared ring would make
        # the Pool rope ops wait on DVE's tile releases (serializing them)
        tmppool = ctx.enter_context(tc.tile_pool(name="tmppool", bufs=4))
        tmppoolp = ctx.enter_context(tc.tile_pool(name="tmppoolp", bufs=4))
        recpool = ctx.enter_context(tc.tile_pool(name="recpool", bufs=1))
        atpool = ctx.enter_context(tc.tile_pool(name="atpool", bufs=1))
        a8pool = ctx.enter_context(tc.tile_pool(name="a8pool", bufs=2))
        wostg = ctx.enter_context(tc.tile_pool(name="wostg", bufs=3))
        ps8 = ctx.enter_context(tc.tile_pool(name="ps8", bufs=8, space="PSUM"))

        # resident weights
        w8t = wres.tile([128, NDC * WROW], F8)
        wrt = wres.tile([128, HPC * 2 * D], F8)

        # constants
        cosT = const.tile([128, S], BF16)
        sinT = const.tile([128, S], BF16)
        dmask = const.tile([128, 256], BF16)
        ones_t = const.tile([128, 128], F32R)
        ident = const.tile([128, 128], F32R)

        def load_consts():
            nc.sync.dma_start(cosT[0:64, :], cosq.ap())
            nc.sync.dma_start(cosT[64:128, :], cosq.ap())
            nc.sync.dma_start(sinT[0:64, :], sinq.ap())
            nc.sync.dma_start(sinT[64:128, :], sinq.ap())
            nc.sync.dma_start(dmask[:], diagm.ap())
            nc.sync.dma_start(ones_t[:], ones4.ap())

        def dma_w8(g, h0=0.0, h1=1.0):
            a = int((g + h0) * GRP * 128)
            b = int((g + h1) * GRP * 128)
            nc.sync.dma_start(
                w8t[:, a * WROW // 128 : b * WROW // 128].rearrange(
                    "p (c x) -> p c x", x=WROW
                ),
                w8.ap()[a:b, :].rearrange("(c p) x -> p c x", p=128),
            )

        # persistent q0..q3,k slabs (transposed [dim, seq]) + transposed V
        qkv = [
            [slabs.tile([128, SQ], F32R, name=f"qkv{s}_{i}") for i in range(5)]
            for s in range(NSQ)
        ]
        vt_s = [slabs.tile([128, SQ], F32R, name=f"vt{s}") for s in range(NSQ)]

        # ---- fused QKV projection: fp8 DoubleRow, PSUM-resident ----
        def proj(s, with_w=False, tail_hook=None):
            ps = [
                ps8.tile([128, SQ], F32, tag="ps", name=f"proj{s}_{nt}")
                for nt in range(NSLAB)
            ]
            started = [False] * NSLAB
            HG = GRP // 2        # chunks per x half-group tile
            for g in range(NG):
                for hf in range(2):
                    # the very first half-group streams w/x in single chunks
                    # so the PE starts as soon as the first chunk lands
                    first_hg = with_w and g == 0 and hf == 0
                    subs = (0.0, 0.25, 0.5, 1.0) if first_hg else (0.0, 1.0)
                    xg = xpool.tile([128, HG * 2 * SQ], F8, tag="x")
                    base = g * GRP + hf * HG     # absolute first chunk
                    for hh in range(len(subs) - 1):
                        c0 = int(subs[hh] * HG)
                        c1 = int(subs[hh + 1] * HG)
                        if with_w:
                            dma_w8(g, (hf * HG + c0) / GRP, (hf * HG + c1) / GRP)
                        nc.sync.dma_start(
                            xg[:, c0 * 2 * SQ : c1 * 2 * SQ].rearrange(
                                "p (c n) -> p c n", n=2 * SQ
                            ),
                            x8.ap()[
                                s * D + (base + c0) * 128 : s * D
                                + (base + c1) * 128,
                                :,
                            ].rearrange("(c p) x -> p c x", p=128),
                        )
                        wv_g = w8t[:, base * WROW : (base + HG) * WROW]
                        w_ch = wv_g.rearrange("p (c x) -> p c x", x=WROW)
                        w_sl = wv_g.rearrange(
                            "p (c t x) -> p c t x", t=2, x=NSLAB * 128
                        )
                        x_pr = xg[:].rearrange("p (c n) -> p c n", n=2 * SQ)
                        x_sl = xg[:].rearrange(
                            "p (c t n) -> p c t n", t=2, n=SQ
                        )
                        for nt in range(NSLAB):
                            co = nt * 128
                            # emit cross terms first (each needs only one
                            # chunk, letting the first matmuls start before
                            # the pair's second chunk lands)
                            last = g == NG - 1 and hf == 1 and c1 == HG
                            for dd in range(c0, c1):
                                # cross terms: (w_hi, w_lo) @ (x_lo, x_hi)
                                nc.tensor.matmul(
                                    ps[nt][:],
                                    w_sl[:, dd, :, co : co + 128],
                                    x_sl[:, dd],
                                    start=not started[nt],
                                    stop=False,
                                    perf_mode=DR,
                                )
                                started[nt] = True
                            for k in range(c0 // 2, (c1 + 1) // 2):
                                # hi@hi over chunk pair (2k, 2k+1) of the
                                # half-group; emitted once both chunks of
                                # the pair are covered
                                if 2 * k + 2 > c1:
                                    continue
                                nc.tensor.matmul(
                                    ps[nt][:],
                                    w_ch[:, 2 * k : 2 * k + 2, co : co + 128],
                                    x_pr[:, 2 * k : 2 * k + 2, SQ : 2 * SQ],
                                    start=False,
                                    stop=(last and 2 * k + 2 == HG),
                                    perf_mode=DR,
                                )
            # V first (its transpose is on the PE critical path), then the
            # next attention chunk's pre-ramp, then the remaining copies
            # spread over ACT/DVE so banks release in parallel and no
            # cross-engine ordering inversions appear.
            vs = vtmp.tile([128, SQ], F32R, tag="v")
            nc.scalar.copy(vs[:], ps[5][:])
            if tail_hook is not None:
                tail_hook()
            for tt in range(4):
                tp = ps8.tile([128, 128], F32R, tag="ps", name=f"vtp{s}_{tt}")
                nc.tensor.transpose(
                    tp[:], vs[:, tt * 128 : (tt + 1) * 128], ident[:]
                )
                nc.scalar.copy(vt_s[s][:, tt * 128 : (tt + 1) * 128], tp[:])
            nc.scalar.copy(qkv[s][0][:], ps[0][:])
            nc.scalar.copy(qkv[s][1][:], ps[1][:])
            nc.vector.tensor_copy(qkv[s][2][:], ps[2][:])
            nc.vector.tensor_copy(qkv[s][3][:], ps[3][:])
            nc.scalar.copy(qkv[s][4][:], ps[4][:])

        def rope(s, dve_slabs=(4, 0)):
            cs_lo = cosT[0:64, s * SQ : (s + 1) * SQ]
            cs_hi = cosT[64:128, s * SQ : (s + 1) * SQ]
            sn_lo = sinT[0:64, s * SQ : (s + 1) * SQ]
            sn_hi = sinT[64:128, s * SQ : (s + 1) * SQ]
            for nt in (4, 0, 1, 2, 3):
                dve = nt in dve_slabs
                eng = nc.vector if dve else nc.gpsimd
                pool = tmppool if dve else tmppoolp
                tl = qkv[s][nt]
                lo = tl[0:64, :]
                hi = tl[64:128, :]
                m1 = pool.tile([64, SQ], F32, tag="t")
                m2 = pool.tile([64, SQ], F32, tag="t")
                m3 = pool.tile([64, SQ], F32, tag="t")
                m4 = pool.tile([64, SQ], F32, tag="t")
                eng.tensor_mul(m1[:], lo, cs_lo)
                eng.tensor_mul(m2[:], hi, sn_hi)
                eng.tensor_mul(m3[:], lo, sn_lo)
                eng.tensor_mul(m4[:], hi, cs_hi)
                eng.tensor_sub(hi, m1[:], m2[:])   # rotated low half
                eng.tensor_add(lo, m3[:], m4[:])   # rotated high half

        def ktile(t):
            return qkv[t // 4][4][:, (t % 4) * 128 : (t % 4) * 128 + 128]

        def vtile(t):
            return vt_s[t // 4][:, (t % 4) * 128 : (t % 4) * 128 + 128]

        attout8 = {}

        # ---- attention (flash, transposed scores, causal block skip) ----
        # software-pipelined: the scores matmul + exp for tile i+L issue
        # before the av/den matmuls of tile i, so the PE never waits on the
        # ACT exp latency.
        def att_make(c, L=3, npre=6):
            """Returns (pre, rest): pre emits the first `npre` sc/exp chains
            (callable from inside the preceding proj phase, using the spare
            PSUM banks); rest emits everything else."""
            ntiles = 4 * c + 4
            avden = {}
            state = {"a8v": None}
            pend = []
            stream = [(h, t) for h in range(HPC) for t in range(ntiles)]

            def finalize(h):
                a8v = state["a8v"]
                av, den = avden.pop(h)
                rec = recpool.tile([128, SQ], F32, tag="rec")
                nc.vector.reciprocal(rec[:], den[:])
                t_f = atpool.tile([128, SQ], F32, tag="t")
                nc.vector.tensor_mul(t_f[:], av[:], rec[:])
                nc.vector.tensor_copy(a8v[:, h, 0, :], t_f[:])  # hi (fp8 cast)
                nc.vector.tensor_sub(a8v[:, h, 1, :], t_f[:], a8v[:, h, 0, :])

            def emit_sc(h, t):
                j = t - 4 * c
                # diagonal tiles: queries below the causal frontier are all
                # masked; shrink the moving free dim (kept >= 256 for fp32r)
                qo = 0 if j < 1 else (128 if j == 1 else 256)
                fr = SQ - qo
                sc = ps8.tile([128, fr], F32, tag="ps", name=f"sc{h}_{c}_{t}")
                nc.tensor.matmul(
                    sc[:], ktile(t), qkv[c][h][:, qo:SQ], start=True, stop=True
                )
                if j >= 0:
                    # only the 128-wide causal boundary block needs masking
                    # (plus one fully-masked block for j=3 whose q-slice
                    # starts below the frontier)
                    if j == 3:
                        nc.vector.tensor_add(
                            sc[:, 0:256], sc[:, 0:256], dmask[:, 0:256]
                        )
                    else:
                        nc.vector.tensor_add(
                            sc[:, 0:128], sc[:, 0:128], dmask[:, 128:256]
                        )
                pt = ptpool.tile([128, fr], F32R, tag="pt")
                nc.scalar.activation(pt[:], sc[:], Exp)
                pend.append((h, t, qo, pt))

            def pre():
                for h, t in stream[:npre]:
                    emit_sc(h, t)

            def rest():
                a8 = a8pool.tile(
                    [128, HPC * 2 * SQ], F8, tag="a8", name=f"a8_{c}"
                )
                attout8[c] = a8
                state["a8v"] = a8[:].rearrange("p (h t n) -> p h t n", h=HPC, t=2)
                for h, t in stream[npre:]:
                    emit_sc(h, t)
                    if len(pend) > L:
                        emit_avden(c, ntiles, avden, pend.pop(0), finalize)
                while pend:
                    emit_avden(c, ntiles, avden, pend.pop(0), finalize)

            return pre, rest

        def emit_avden(c, ntiles, avden, item, finalize):
            h, t, qo, pt = item
            if t == 0:
                avden[h] = (
                    ps8.tile([128, SQ], F32, tag="ps", name=f"av{h}_{c}"),
                    ps8.tile([128, SQ], F32, tag="ps", name=f"den{h}_{c}"),
                )
            av, den = avden[h]
            last = t == ntiles - 1
            nc.tensor.matmul(
                av[:, qo:SQ], vtile(t), pt[:], start=(t == 0), stop=last
            )
            nc.tensor.matmul(
                den[:, qo:SQ], ones_t[:], pt[:], start=(t == 0), stop=last
            )
            if last:
                finalize(h)

        # ---- wo projection: fp8 DoubleRow, bf16 staged output ----
        def wo(c, last=False):
            a8 = attout8.pop(c)
            a8v = a8[:].rearrange("p (h t n) -> p h t n", h=HPC, t=2)
            wrv = wrt[:].rearrange("p (h t n) -> p h t n", h=HPC, t=2)
            for m in range(4):
                mo = m * 128
                # final row of the kernel drains per-tile so the last DMA is
                # small (shorter end-of-kernel tail)
                fine = last and m == 3
                for jq in range(2):
                    st = wostg.tile([128, 4 * SQ], BF16, tag="st")
                    for jj in range(4):
                        j = jq * 4 + jj
                        po = ps8.tile(
                            [128, SQ], F32, tag="ps", name=f"po{c}_{m}_{j}"
                        )
                        for p in range(HPC // 2):
                            # hi@hi over head pair (2p, 2p+1)
                            nc.tensor.matmul(
                                po[:],
                                a8v[:, 2 * p : 2 * p + 2, 0, mo : mo + 128],
                                wrv[:, 2 * p : 2 * p + 2, 1, j * SQ : (j + 1) * SQ],
                                start=(p == 0),
                                stop=False,
                                perf_mode=DR,
                            )
                        for h in range(HPC):
                            # cross: (a_hi, a_lo) @ (wo_lo, wo_hi)
                            nc.tensor.matmul(
                                po[:],
                                a8v[:, h, :, mo : mo + 128],
                                wrv[:, h, :, j * SQ : (j + 1) * SQ],
                                start=False,
                                stop=(h == HPC - 1),
                                perf_mode=DR,
                            )
                        # GPSIMD cannot read PSUM; alternate ACT/DVE
                        eng = (nc.scalar.copy, nc.vector.tensor_copy)[
                            (m * 8 + j) % 2
                        ]
                        eng(st[:, jj * SQ : (jj + 1) * SQ], po[:])
                        if fine:
                            nc.sync.dma_start(
                                out.ap()[
                                    (4 * c + m) * 128 : (4 * c + m + 1) * 128,
                                    j * SQ : (j + 1) * SQ,
                                ],
                                st[:, jj * SQ : (jj + 1) * SQ],
                            )
                    if not fine:
                        nc.sync.dma_start(
                            out.ap()[
                                (4 * c + m) * 128 : (4 * c + m + 1) * 128,
                                jq * 4 * SQ : (jq + 1) * 4 * SQ,
                            ],
                            st[:],
                        )

        # ---- schedule ----
        # ident MUST be emitted before proj(0): its V-transposes read it,
        # and a read emitted before the write would consume garbage
        nc.sync.dma_start(ident[:], identd.ap())
        proj(0, with_w=True)
        load_consts()
        rope(0)
        a0_pre, a0_rest = att_make(0)
        proj(1, tail_hook=a0_pre)
        a0_rest()
        rope(1)
        a1_pre, a1_rest = att_make(1)
        proj(2, tail_hook=a1_pre)
        nc.sync.dma_start(wrt[:], wor8.ap())
        a1_rest()
        wo(0)
        rope(2)
        a2_pre, a2_rest = att_make(2)
        proj(3, tail_hook=a2_pre)
        a2_rest()
        wo(1)
        rope(3)
        a3_pre, a3_rest = att_make(3)
        a3_pre()
        a3_rest()
        wo(2)
        wo(3, last=True)


def _host_prep(x, wq, wk, wv, wo, freqs_cos, freqs_sin):
    """Build the 8 per-core input maps (fp8 hi/lo splits, rescaled)."""
    perm = np.concatenate([np.arange(0, HD, 2), np.arange(1, HD, 2)])
    xt = np.ascontiguousarray(x.reshape(S, D).T)
    xhi = xt.astype(E4)
    xlo = (xt - xhi.astype(np.float32)).astype(E4)
    # s-chunk-major, (lo, hi) slots: [NSQ, D, 2, SQ] -> [NSQ * D, 2 * SQ]
    x8 = np.ascontiguousarray(
        np.stack(
            [
                xlo.reshape(D, NSQ, SQ).transpose(1, 0, 2),
                xhi.reshape(D, NSQ, SQ).transpose(1, 0, 2),
            ],
            axis=2,
        ).reshape(NSQ * D, 2 * SQ)
    )
    cosq = np.ascontiguousarray((freqs_cos.T.astype(np.float32) * BETA).astype(ml_dtypes.bfloat16))
    sinq = np.ascontiguousarray((freqs_sin.T.astype(np.float32) * BETA).astype(ml_dtypes.bfloat16))
    kk = np.arange(128)[:, None]
    qq = np.arange(128)[None, :]
    tri = np.where(kk <= qq, 0.0, -1e9).astype(np.float32)
    diagm = np.concatenate(
        [np.full((128, 128), -1e9, np.float32), tri], axis=1
    ).astype(ml_dtypes.bfloat16)
    ones4 = np.full((128, 128), CONES, np.float32)
    ident = np.eye(128, dtype=np.float32)

    in_maps = []
    for c in range(NCORES):
        wq_c = (
            wq[:, (HPC * c) * HD : (HPC * c + HPC) * HD]
            .reshape(D, HPC, HD)[:, :, perm]
            .reshape(D, HPC * HD)
        )
        wk_c = wk[:, c * HD : (c + 1) * HD][:, perm]
        wv_c = wv[:, c * HD : (c + 1) * HD]
        wcat = np.concatenate([wq_c, wk_c, wv_c], axis=1) * WSC  # [D, 768]
        whi = wcat.astype(E4)
        wlo = (wcat - whi.astype(np.float32)).astype(E4)
        w8 = np.ascontiguousarray(
            np.stack([whi, wlo], axis=1).reshape(D, 2 * NSLAB * 128)
        )  # (hi, lo)
        wo_c = (
            wo[(HPC * c) * HD : (HPC * c + HPC) * HD, :].reshape(HPC, 128, D)
            * WSC
        )
        wo_hd = wo_c.transpose(1, 0, 2)  # [128 hd, HPC, D]
        whi_o = wo_hd.astype(E4)
        wlo_o = (wo_hd - whi_o.astype(np.float32)).astype(E4)
        wor8 = np.ascontiguousarray(
            np.stack([wlo_o, whi_o], axis=2).reshape(128, HPC * 2 * D)
        )  # (lo, hi)
        in_maps.append(
            {
                "w8": w8,
                "x8": x8,
                "wor8": wor8,
                "cosq": cosq,
                "sinq": sinq,
                "diagm": diagm,
                "ones4": ones4,
                "identd": ident,
            }
        )
    return in_maps


def _numpy_fallback(x, wq, wk, wv, wo, freqs_cos, freqs_sin, mask):
    """Exact reference math in numpy (used only for non-causal masks)."""
    bsz = x.shape[0]
    n_rep = H // H_KV
    xq = (x.reshape(-1, D) @ wq).reshape(bsz, S, H, HD)
    xk = (x.reshape(-1, D) @ wk).reshape(bsz, S, H_KV, HD)
    xv = (x.reshape(-1, D) @ wv).reshape(bsz, S, H_KV, HD)

    def rope(t):
        t0, t1 = t[..., 0::2], t[..., 1::2]
        c = freqs_cos[None, :, None, :]
        s = freqs_sin[None, :, None, :]
        o0 = t0 * c - t1 * s
        o1 = t0 * s + t1 * c
        return np.stack([o0, o1], axis=-1).reshape(t.shape)

    xq, xk = rope(xq), rope(xk)
    keys = np.repeat(xk, n_rep, axis=2)
    values = np.repeat(xv, n_rep, axis=2)
    scores = np.einsum("bqhd,bkhd->bhqk", xq, keys) / math.sqrt(HD)
    scores = scores + mask[:, :, -S:, -S:]
    scores = scores - scores.max(axis=-1, keepdims=True)
    e = np.exp(scores)
    attn = e / e.sum(axis=-1, keepdims=True)
    o = np.einsum("bhqk,bkhd->bqhd", attn, values).reshape(bsz, S, H * HD)
    return (o @ wo).astype(np.float32)


def kernel(**inputs):
    x = np.asarray(inputs["x"], dtype=np.float32)
    wq = np.asarray(inputs["wq"], dtype=np.float32)
    wk = np.asarray(inputs["wk"], dtype=np.float32)
    wv = np.asarray(inputs["wv"], dtype=np.float32)
    wo = np.asarray(inputs["wo"], dtype=np.float32)
    fc = np.asarray(inputs["freqs_cos"], dtype=np.float32)
    fs = np.asarray(inputs["freqs_sin"], dtype=np.float32)
    mask = np.asarray(inputs["mask"], dtype=np.float32)

    causal = np.triu(np.full((S, S), -1e9, dtype=np.float32), k=1)[None, None]
    if x.shape != (1, S, D) or not np.array_equal(mask, causal):
        return _numpy_fallback(x, wq, wk, wv, wo, fc, fs, mask)

    if "nc" not in _NC_CACHE:
        _NC_CACHE["nc"] = _build_nc()
    nc = _NC_CACHE["nc"]
    in_maps = _host_prep(x[0], wq, wk, wv, wo, fc, fs)
    _log("launching on 8 cores (compile on first call + transfers)")
    res = run_bass_kernel_spmd(nc, in_maps, core_ids=list(range(NCORES)))
    _log("run complete")
    full = np.zeros((S, D), np.float32)
    for r in res.results:
        full += r["out"].astype(np.float32)
    full /= OUT_DIV
    return full.reshape(1, S, D)


# revision 7
# speedup vs baseline: 1.0351x; 1.0003x over previous
"""Trainium2 Bass kernel for GQA attention (B=1, S=2048, D=4096, H=32, H_KV=8, HD=128).

Sharding: tensor-parallel over heads, 8 cores; core c owns Q heads 4c..4c+3 and
KV head c.  Each core computes a partial [S, D] output (wo row-shard); the host
sums the partials.

Per-core kernel v2 (fp8-DoubleRow compensated projections):
  - QKV projection and the wo output projection run as fp8e4m3 DoubleRow
    matmuls (cost-model 0.5 cycles/row, 256-deep contraction per
    instruction).  Full fp32-level accuracy is recovered with a hi/lo
    split: w = w_hi + w_lo, x = x_hi + x_lo (host-side, power-of-2
    rescaled so both splits stay in fp8's normal range), computing
    w_hi@x_hi (chunk-paired) + (w_hi@x_lo + w_lo@x_hi) (slot-paired in a
    single DoubleRow op).  The dropped w_lo@x_lo term is ~1e-3 relative.
  - All scale factors fold into free spots: the RoPE cos/sin tables carry
    the score scale, the softmax-denominator ones-matrix carries the V
    rescale, and the host divides the summed output by a single constant.
  - QKV accumulates entirely in PSUM (6 banks), no SBUF folding; weights
    and x stream once (fp8, half traffic).
  - Attention: flash-style transposed scores (fp32r), software-pipelined
    with a 3-tile scores/exp lookahead (plus a pre-ramp of 6 tiles emitted
    inside the preceding projection phase).  Causal handling skips
    above-diagonal key tiles, shrinks diagonal-tile matmuls to the live
    query range (>=256 free for fp32r full rate), and masks only the
    128-wide causal boundary block.  The softmax denominator comes from a
    4.0-matrix matmul; normalization and the attout hi/lo fp8 split run on
    DVE; RoPE is split DVE/GpSimd with per-engine scratch pools.
  - Output staged to bf16 in SBUF (ACT/DVE alternating copies, quad-batched
    DMAs; per-tile DMAs on the final row), host sums partials in fp32.
  - PE phase order: p0 p1 a0 p2 a1 w0 p3 a2 w1 a3 w2 w3 so RoPE, exp,
    attout finalization and output DMA all hide under PE work.  TimelineSim:
    317647 ns (baseline 468245 ns), PE busy 299 us (94%).
"""

import math
import os
import sys
import time

import numpy as np
import ml_dtypes

E4 = ml_dtypes.float8_e4m3fn


def _log(msg):
    if os.environ.get("KERNEL_QUIET"):
        return
    print(f"[kernel {time.strftime('%H:%M:%S')}] {msg}", file=sys.stderr, flush=True)


import concourse.bass as bass
import concourse.tile as tile
from concourse import bacc, mybir
from concourse.bass_utils import run_bass_kernel_spmd

S, D = 2048, 4096
H, H_KV, HD = 32, 8, 128
NCORES = 8
HPC = H // NCORES            # 4 Q heads per core
NSLAB = 6                    # q0..q3, k, v slabs of 128 cols
SQ = 512
NSQ = S // SQ                # 4
NKT = S // 128               # 16 key tiles
NDC = D // 128               # 32 contraction chunks
GRP = 8                      # chunks per DMA group
NG = NDC // GRP              # 4 groups
F32 = mybir.dt.float32
F32R = mybir.dt.float32r
F8 = mybir.dt.float8e4
BF16 = mybir.dt.bfloat16
DR = mybir.MatmulPerfMode.DoubleRow
Exp = mybir.ActivationFunctionType.Exp

WSC = 64.0                       # weight rescale for fp8 range
BETA = 1.0 / (WSC * 128.0 ** 0.25)   # rope cos/sin scale (per q and k)
CONES = 4.0                      # denominator matrix value -> attout = 16x true
OUT_DIV = (WSC / CONES) * WSC    # host divides summed output by this (1024)

_NC_CACHE = {}


def _build_nc():
    nc = bacc.Bacc(
        "TRN2", target_bir_lowering=False, debug=False, enable_asserts=False
    )
    w8 = nc.dram_tensor("w8", [D, 2 * NSLAB * 128], F8, kind="ExternalInput")
    # s-chunk-major: [NSQ * D, (lo, hi) * SQ] so each (g, s) load is 3-dim
    x8 = nc.dram_tensor("x8", [NSQ * D, 2 * SQ], F8, kind="ExternalInput")
    wor8 = nc.dram_tensor("wor8", [128, HPC * 2 * D], F8, kind="ExternalInput")
    cosq = nc.dram_tensor("cosq", [64, S], BF16, kind="ExternalInput")
    sinq = nc.dram_tensor("sinq", [64, S], BF16, kind="ExternalInput")
    diagm = nc.dram_tensor("diagm", [128, 256], BF16, kind="ExternalInput")
    ones4 = nc.dram_tensor("ones4", [128, 128], F32R, kind="ExternalInput")
    identd = nc.dram_tensor("identd", [128, 128], F32R, kind="ExternalInput")
    out = nc.dram_tensor("out", [S, D], BF16, kind="ExternalOutput")

    _log("emitting IR")
    with tile.TileContext(nc) as tc:
        _emit(tc, w8, x8, wor8, cosq, sinq, diagm, ones4, identd, out)
    _log("bacc compile")
    nc.compile()
    _log("bass module ready")
    return nc


def _emit(tc, w8, x8, wor8, cosq, sinq, diagm, ones4, identd, out):
    from contextlib import ExitStack

    nc = tc.nc
    WROW = 2 * NSLAB * 128       # 1536 fp8 cols per d-chunk of w8
    with ExitStack() as ctx:
        const = ctx.enter_context(tc.tile_pool(name="const", bufs=1))
        wres = ctx.enter_context(tc.tile_pool(name="wres", bufs=1))
        slabs = ctx.enter_context(tc.tile_pool(name="slabs", bufs=1))
        xpool = ctx.enter_context(tc.tile_pool(name="xpool", bufs=4))
        vtmp = ctx.enter_context(tc.tile_pool(name="vtmp", bufs=1))
        ptpool = ctx.enter_context(tc.tile_pool(name="ptpool", bufs=6))
        # separate rope scratch rings per engine: a shared ring would make
        # the Pool rope ops wait on DVE's tile releases (serializing them)
        tmppool = ctx.enter_context(tc.tile_pool(name="tmppool", bufs=4))
        tmppoolp = ctx.enter_context(tc.tile_pool(name="tmppoolp", bufs=4))
        recpool = ctx.enter_context(tc.tile_pool(name="recpool", bufs=1))
        atpool = ctx.enter_context(tc.tile_pool(name="atpool", bufs=1))
        a8pool = ctx.enter_context(tc.tile_pool(name="a8pool", bufs=2))
        wostg = ctx.enter_context(tc.tile_pool(name="wostg", bufs=3))
        ps8 = ctx.enter_context(tc.tile_pool(name="ps8", bufs=8, space="PSUM"))

        # resident weights
        w8t = wres.tile([128, NDC * WROW], F8)
        wrt = wres.tile([128, HPC * 2 * D], F8)

        # constants
        cosT = const.tile([128, S], BF16)
        sinT = const.tile([128, S], BF16)
        dmask = const.tile([128, 256], BF16)
        ones_t = const.tile([128, 128], F32R)
        ident = const.tile([128, 128], F32R)

        def load_consts():
            nc.sync.dma_start(cosT[0:64, :], cosq.ap())
            nc.sync.dma_start(cosT[64:128, :], cosq.ap())
            nc.sync.dma_start(sinT[0:64, :], sinq.ap())
            nc.sync.dma_start(sinT[64:128, :], sinq.ap())
            nc.sync.dma_start(dmask[:], diagm.ap())
            nc.sync.dma_start(ones_t[:], ones4.ap())

        def dma_w8(g, h0=0.0, h1=1.0):
            a = int((g + h0) * GRP * 128)
            b = int((g + h1) * GRP * 128)
            nc.sync.dma_start(
                w8t[:, a * WROW // 128 : b * WROW // 128].rearrange(
                    "p (c x) -> p c x", x=WROW
                ),
                w8.ap()[a:b, :].rearrange("(c p) x -> p c x", p=128),
            )

        # persistent q0..q3,k slabs (transposed [dim, seq]) + transposed V
        qkv = [
            [slabs.tile([128, SQ], F32R, name=f"qkv{s}_{i}") for i in range(5)]
            for s in range(NSQ)
        ]
        vt_s = [slabs.tile([128, SQ], F32R, name=f"vt{s}") for s in range(NSQ)]

        # ---- fused QKV projection: fp8 DoubleRow, PSUM-resident ----
        def proj(s, with_w=False, tail_hook=None):
            ps = [
                ps8.tile([128, SQ], F32, tag="ps", name=f"proj{s}_{nt}")
                for nt in range(NSLAB)
            ]
            started = [False] * NSLAB
            HG = GRP // 2        # chunks per x half-group tile
            for g in range(NG):
                for hf in range(2):
                    # the very first half-group streams w/x in single chunks
                    # so the PE starts as soon as the first chunk lands
                    first_hg = with_w and g == 0 and hf == 0
                    subs = (0.0, 0.25, 0.5, 1.0) if first_hg else (0.0, 1.0)
                    xg = xpool.tile([128, HG * 2 * SQ], F8, tag="x")
                    base = g * GRP + hf * HG     # absolute first chunk
                    for hh in range(len(subs) - 1):
                        c0 = int(subs[hh] * HG)
                        c1 = int(subs[hh + 1] * HG)
                        if with_w:
                            dma_w8(g, (hf * HG + c0) / GRP, (hf * HG + c1) / GRP)
                        nc.sync.dma_start(
                            xg[:, c0 * 2 * SQ : c1 * 2 * SQ].rearrange(
                                "p (c n) -> p c n", n=2 * SQ
                            ),
                            x8.ap()[
                                s * D + (base + c0) * 128 : s * D
                                + (base + c1) * 128,
                                :,
                            ].rearrange("(c p) x -> p c x", p=128),
                        )
                        wv_g = w8t[:, base * WROW : (base + HG) * WROW]
                        w_ch = wv_g.rearrange("p (c x) -> p c x", x=WROW)
                        w_sl = wv_g.rearrange(
                            "p (c t x) -> p c t x", t=2, x=NSLAB * 128
                        )
                        x_pr = xg[:].rearrange("p (c n) -> p c n", n=2 * SQ)
                        x_sl = xg[:].rearrange(
                            "p (c t n) -> p c t n", t=2, n=SQ
                        )
                        for nt in range(NSLAB):
                            co = nt * 128
                            # emit cross terms first (each needs only one
                            # chunk, letting the first matmuls start before
                            # the pair's second chunk lands)
                            last = g == NG - 1 and hf == 1 and c1 == HG
                            for dd in range(c0, c1):
                                # cross terms: (w_hi, w_lo) @ (x_lo, x_hi)
                                nc.tensor.matmul(
                                    ps[nt][:],
                                    w_sl[:, dd, :, co : co + 128],
                                    x_sl[:, dd],
                                    start=not started[nt],
                                    stop=False,
                                    perf_mode=DR,
                                )
                                started[nt] = True
                            for k in range(c0 // 2, (c1 + 1) // 2):
                                # hi@hi over chunk pair (2k, 2k+1) of the
                                # half-group; emitted once both chunks of
                                # the pair are covered
                                if 2 * k + 2 > c1:
                                    continue
                                nc.tensor.matmul(
                                    ps[nt][:],
                                    w_ch[:, 2 * k : 2 * k + 2, co : co + 128],
                                    x_pr[:, 2 * k : 2 * k + 2, SQ : 2 * SQ],
                                    start=False,
                                    stop=(last and 2 * k + 2 == HG),
                                    perf_mode=DR,
                                )
            # V first (its transpose is on the PE critical path), then the
            # next attention chunk's pre-ramp, then the remaining copies
            # spread over ACT/DVE so banks release in parallel and no
            # cross-engine ordering inversions appear.
            vs = vtmp.tile([128, SQ], F32R, tag="v")
            nc.scalar.copy(vs[:], ps[5][:])
            if tail_hook is not None:
                tail_hook()
            for tt in range(4):
                tp = ps8.tile([128, 128], F32R, tag="ps", name=f"vtp{s}_{tt}")
                nc.tensor.transpose(
                    tp[:], vs[:, tt * 128 : (tt + 1) * 128], ident[:]
                )
                nc.scalar.copy(vt_s[s][:, tt * 128 : (tt + 1) * 128], tp[:])
            nc.scalar.copy(qkv[s][0][:], ps[0][:])
            nc.scalar.copy(qkv[s][1][:], ps[1][:])
            nc.vector.tensor_copy(qkv[s][2][:], ps[2][:])
            nc.vector.tensor_copy(qkv[s][3][:], ps[3][:])
            nc.scalar.copy(qkv[s][4][:], ps[4][:])

        def rope(s, dve_slabs=(4, 0)):
            cs_lo = cosT[0:64, s * SQ : (s + 1) * SQ]
            cs_hi = cosT[64:128, s * SQ : (s + 1) * SQ]
            sn_lo = sinT[0:64, s * SQ : (s + 1) * SQ]
            sn_hi = sinT[64:128, s * SQ : (s + 1) * SQ]
            for nt in (4, 0, 1, 2, 3):
                dve = nt in dve_slabs
                eng = nc.vector if dve else nc.gpsimd
                pool = tmppool if dve else tmppoolp
                tl = qkv[s][nt]
                lo = tl[0:64, :]
                hi = tl[64:128, :]
                m1 = pool.tile([64, SQ], F32, tag="t")
                m2 = pool.tile([64, SQ], F32, tag="t")
                m3 = pool.tile([64, SQ], F32, tag="t")
                m4 = pool.tile([64, SQ], F32, tag="t")
                eng.tensor_mul(m1[:], lo, cs_lo)
                eng.tensor_mul(m2[:], hi, sn_hi)
                eng.tensor_mul(m3[:], lo, sn_lo)
                eng.tensor_mul(m4[:], hi, cs_hi)
                eng.tensor_sub(hi, m1[:], m2[:])   # rotated low half
                eng.tensor_add(lo, m3[:], m4[:])   # rotated high half

        def ktile(t):
            return qkv[t // 4][4][:, (t % 4) * 128 : (t % 4) * 128 + 128]

        def vtile(t):
            return vt_s[t // 4][:, (t % 4) * 128 : (t % 4) * 128 + 128]

        attout8 = {}

        # ---- attention (flash, transposed scores, causal block skip) ----
        # software-pipelined: the scores matmul + exp for tile i+L issue
        # before the av/den matmuls of tile i, so the PE never waits on the
        # ACT exp latency.
        def att_make(c, L=3, npre=6):
            """Returns (pre, rest): pre emits the first `npre` sc/exp chains
            (callable from inside the preceding proj phase, using the spare
            PSUM banks); rest emits everything else."""
            ntiles = 4 * c + 4
            avden = {}
            state = {"a8v": None}
            pend = []
            stream = [(h, t) for h in range(HPC) for t in range(ntiles)]

            def finalize(h):
                a8v = state["a8v"]
                av, den = avden.pop(h)
                rec = recpool.tile([128, SQ], F32, tag="rec")
                nc.vector.reciprocal(rec[:], den[:])
                t_f = atpool.tile([128, SQ], F32, tag="t")
                nc.vector.tensor_mul(t_f[:], av[:], rec[:])
                nc.vector.tensor_copy(a8v[:, h, 0, :], t_f[:])  # hi (fp8 cast)
                nc.vector.tensor_sub(a8v[:, h, 1, :], t_f[:], a8v[:, h, 0, :])

            def emit_sc(h, t):
                j = t - 4 * c
                # diagonal tiles: queries below the causal frontier are all
                # masked; shrink the moving free dim (kept >= 256 for fp32r)
                qo = 0 if j < 1 else (128 if j == 1 else 256)
                fr = SQ - qo
                sc = ps8.tile([128, fr], F32, tag="ps", name=f"sc{h}_{c}_{t}")
                nc.tensor.matmul(
                    sc[:], ktile(t), qkv[c][h][:, qo:SQ], start=True, stop=True
                )
                if j >= 0:
                    # only the 128-wide causal boundary block needs masking
                    # (plus one fully-masked block for j=3 whose q-slice
                    # starts below the frontier)
                    if j == 3:
                        nc.vector.tensor_add(
                            sc[:, 0:256], sc[:, 0:256], dmask[:, 0:256]
                        )
                    else:
                        nc.vector.tensor_add(
                            sc[:, 0:128], sc[:, 0:128], dmask[:, 128:256]
                        )
                pt = ptpool.tile([128, fr], F32R, tag="pt")
                nc.scalar.activation(pt[:], sc[:], Exp)
                pend.append((h, t, qo, pt))

            def pre():
                for h, t in stream[:npre]:
                    emit_sc(h, t)

            def rest():
                a8 = a8pool.tile(
                    [128, HPC * 2 * SQ], F8, tag="a8", name=f"a8_{c}"
                )
                attout8[c] = a8
                state["a8v"] = a8[:].rearrange("p (h t n) -> p h t n", h=HPC, t=2)
                for h, t in stream[npre:]:
                    emit_sc(h, t)
                    if len(pend) > L:
                        emit_avden(c, ntiles, avden, pend.pop(0), finalize)
                while pend:
                    emit_avden(c, ntiles, avden, pend.pop(0), finalize)

            return pre, rest

        def emit_avden(c, ntiles, avden, item, finalize):
            h, t, qo, pt = item
            if t == 0:
                avden[h] = (
                    ps8.tile([128, SQ], F32, tag="ps", name=f"av{h}_{c}"),
                    ps8.tile([128, SQ], F32, tag="ps", name=f"den{h}_{c}"),
                )
            av, den = avden[h]
            last = t == ntiles - 1
            nc.tensor.matmul(
                den[:, qo:SQ], ones_t[:], pt[:], start=(t == 0), stop=last
            )
            nc.tensor.matmul(
                av[:, qo:SQ], vtile(t), pt[:], start=(t == 0), stop=last
            )
            if last:
                finalize(h)

        # ---- wo projection: fp8 DoubleRow, bf16 staged output ----
        def wo(c, last=False):
            a8 = attout8.pop(c)
            a8v = a8[:].rearrange("p (h t n) -> p h t n", h=HPC, t=2)
            wrv = wrt[:].rearrange("p (h t n) -> p h t n", h=HPC, t=2)
            for m in range(4):
                mo = m * 128
                # final row of the kernel drains per-tile so the last DMA is
                # small (shorter end-of-kernel tail)
                fine = last and m == 3
                for jq in range(2):
                    st = wostg.tile([128, 4 * SQ], BF16, tag="st")
                    for jj in range(4):
                        j = jq * 4 + jj
                        po = ps8.tile(
                            [128, SQ], F32, tag="ps", name=f"po{c}_{m}_{j}"
                        )
                        for p in range(HPC // 2):
                            # hi@hi over head pair (2p, 2p+1)
                            nc.tensor.matmul(
                                po[:],
                                a8v[:, 2 * p : 2 * p + 2, 0, mo : mo + 128],
                                wrv[:, 2 * p : 2 * p + 2, 1, j * SQ : (j + 1) * SQ],
                                start=(p == 0),
                                stop=False,
                                perf_mode=DR,
                            )
                        for h in range(HPC):
                            # cross: (a_hi, a_lo) @ (wo_lo, wo_hi)
                            nc.tensor.matmul(
                                po[:],
                                a8v[:, h, :, mo : mo + 128],
                                wrv[:, h, :, j * SQ : (j + 1) * SQ],
                                start=False,
                                stop=(h == HPC - 1),
                                perf_mode=DR,
                            )
                        # GPSIMD cannot read PSUM; alternate ACT/DVE
                        eng = (nc.scalar.copy, nc.vector.tensor_copy)[
                            (m * 8 + j) % 2
                        ]
                        eng(st[:, jj * SQ : (jj + 1) * SQ], po[:])
                        if fine:
                            nc.sync.dma_start(
                                out.ap()[
                                    (4 * c + m) * 128 : (4 * c + m + 1) * 128,
                                    j * SQ : (j + 1) * SQ,
                                ],
                                st[:, jj * SQ : (jj + 1) * SQ],
                            )
                    if not fine:
                        nc.sync.dma_start(
                            out.ap()[
                                (4 * c + m) * 128 : (4 * c + m + 1) * 128,
                                jq * 4 * SQ : (jq + 1) * 4 * SQ,
                            ],
                            st[:],
                        )

        # ---- schedule ----
        # ident MUST be emitted before proj(0): its V-transposes read it,
        # and a read emitted before the write would consume garbage
        nc.sync.dma_start(ident[:], identd.ap())
        proj(0, with_w=True)
        load_consts()
        rope(0)
        a0_pre, a0_rest = att_make(0)
        proj(1, tail_hook=a0_pre)
        a0_rest()
        rope(1)
        a1_pre, a1_rest = att_make(1)
        proj(2, tail_hook=a1_pre)
        nc.sync.dma_start(wrt[:], wor8.ap())
        a1_rest()
        wo(0)
        rope(2)
        a2_pre, a2_rest = att_make(2)
        proj(3, tail_hook=a2_pre)
        a2_rest()
        wo(1)
        rope(3)
        a3_pre, a3_rest = att_make(3)
        a3_pre()
        a3_rest()
        wo(2)
        wo(3, last=True)


def _host_prep(x, wq, wk, wv, wo, freqs_cos, freqs_sin):
    """Build the 8 per-core input maps (fp8 hi/lo splits, rescaled)."""
    perm = np.concatenate([np.arange(0, HD, 2), np.arange(1, HD, 2)])
    xt = np.ascontiguousarray(x.reshape(S, D).T)
    xhi = xt.astype(E4)
    xlo = (xt - xhi.astype(np.float32)).astype(E4)
    # s-chunk-major, (lo, hi) slots: [NSQ, D, 2, SQ] -> [NSQ * D, 2 * SQ]
    x8 = np.ascontiguousarray(
        np.stack(
            [
                xlo.reshape(D, NSQ, SQ).transpose(1, 0, 2),
                xhi.reshape(D, NSQ, SQ).transpose(1, 0, 2),
            ],
            axis=2,
        ).reshape(NSQ * D, 2 * SQ)
    )
    cosq = np.ascontiguousarray((freqs_cos.T.astype(np.float32) * BETA).astype(ml_dtypes.bfloat16))
    sinq = np.ascontiguousarray((freqs_sin.T.astype(np.float32) * BETA).astype(ml_dtypes.bfloat16))
    kk = np.arange(128)[:, None]
    qq = np.arange(128)[None, :]
    tri = np.where(kk <= qq, 0.0, -1e9).astype(np.float32)
    diagm = np.concatenate(
        [np.full((128, 128), -1e9, np.float32), tri], axis=1
    ).astype(ml_dtypes.bfloat16)
    ones4 = np.full((128, 128), CONES, np.float32)
    ident = np.eye(128, dtype=np.float32)

    in_maps = []
    for c in range(NCORES):
        wq_c = (
            wq[:, (HPC * c) * HD : (HPC * c + HPC) * HD]
            .reshape(D, HPC, HD)[:, :, perm]
            .reshape(D, HPC * HD)
        )
        wk_c = wk[:, c * HD : (c + 1) * HD][:, perm]
        wv_c = wv[:, c * HD : (c + 1) * HD]
        wcat = np.concatenate([wq_c, wk_c, wv_c], axis=1) * WSC  # [D, 768]
        whi = wcat.astype(E4)
        wlo = (wcat - whi.astype(np.float32)).astype(E4)
        w8 = np.ascontiguousarray(
            np.stack([whi, wlo], axis=1).reshape(D, 2 * NSLAB * 128)
        )  # (hi, lo)
        wo_c = (
            wo[(HPC * c) * HD : (HPC * c + HPC) * HD, :].reshape(HPC, 128, D)
            * WSC
        )
        wo_hd = wo_c.transpose(1, 0, 2)  # [128 hd, HPC, D]
        whi_o = wo_hd.astype(E4)
        wlo_o = (wo_hd - whi_o.astype(np.float32)).astype(E4)
        wor8 = np.ascontiguousarray(
            np.stack([wlo_o, whi_o], axis=2).reshape(128, HPC * 2 * D)
        )  # (lo, hi)
        in_maps.append(
            {
                "w8": w8,
                "x8": x8,
                "wor8": wor8,
                "cosq": cosq,
                "sinq": sinq,
                "diagm": diagm,
                "ones4": ones4,
                "identd": ident,
            }
        )
    return in_maps


def _numpy_fallback(x, wq, wk, wv, wo, freqs_cos, freqs_sin, mask):
    """Exact reference math in numpy (used only for non-causal masks)."""
    bsz = x.shape[0]
    n_rep = H // H_KV
    xq = (x.reshape(-1, D) @ wq).reshape(bsz, S, H, HD)
    xk = (x.reshape(-1, D) @ wk).reshape(bsz, S, H_KV, HD)
    xv = (x.reshape(-1, D) @ wv).reshape(bsz, S, H_KV, HD)

    def rope(t):
        t0, t1 = t[..., 0::2], t[..., 1::2]
        c = freqs_cos[None, :, None, :]
        s = freqs_sin[None, :, None, :]
        o0 = t0 * c - t1 * s
        o1 = t0 * s + t1 * c
        return np.stack([o0, o1], axis=-1).reshape(t.shape)

    xq, xk = rope(xq), rope(xk)
    keys = np.repeat(xk, n_rep, axis=2)
    values = np.repeat(xv, n_rep, axis=2)
    scores = np.einsum("bqhd,bkhd->bhqk", xq, keys) / math.sqrt(HD)
    scores = scores + mask[:, :, -S:, -S:]
    scores = scores - scores.max(axis=-1, keepdims=True)
    e = np.exp(scores)
    attn = e / e.sum(axis=-1, keepdims=True)
    o = np.einsum("bhqk,bkhd->bqhd", attn, values).reshape(bsz, S, H * HD)
    return (o @ wo).astype(np.float32)


def kernel(**inputs):
    x = np.asarray(inputs["x"], dtype=np.float32)
    wq = np.asarray(inputs["wq"], dtype=np.float32)
    wk = np.asarray(inputs["wk"], dtype=np.float32)
    wv = np.asarray(inputs["wv"], dtype=np.float32)
    wo = np.asarray(inputs["wo"], dtype=np.float32)
    fc = np.asarray(inputs["freqs_cos"], dtype=np.float32)
    fs = np.asarray(inputs["freqs_sin"], dtype=np.float32)
    mask = np.asarray(inputs["mask"], dtype=np.float32)

    causal = np.triu(np.full((S, S), -1e9, dtype=np.float32), k=1)[None, None]
    if x.shape != (1, S, D) or not np.array_equal(mask, causal):
        return _numpy_fallback(x, wq, wk, wv, wo, fc, fs, mask)

    if "nc" not in _NC_CACHE:
        _NC_CACHE["nc"] = _build_nc()
    nc = _NC_CACHE["nc"]
    in_maps = _host_prep(x[0], wq, wk, wv, wo, fc, fs)
    _log("launching on 8 cores (compile on first call + transfers)")
    res = run_bass_kernel_spmd(nc, in_maps, core_ids=list(range(NCORES)))
    _log("run complete")
    full = np.zeros((S, D), np.float32)
    for r in res.results:
        full += r["out"].astype(np.float32)
    full /= OUT_DIV
    return full.reshape(1, S, D)


# revision 8
# speedup vs baseline: 1.0375x; 1.0023x over previous
"""Trainium2 Bass kernel for GQA attention (B=1, S=2048, D=4096, H=32, H_KV=8, HD=128).

Sharding: tensor-parallel over heads, 8 cores; core c owns Q heads 4c..4c+3 and
KV head c.  Each core computes a partial [S, D] output (wo row-shard); the host
sums the partials.

Per-core kernel v2 (fp8-DoubleRow compensated projections):
  - QKV projection and the wo output projection run as fp8e4m3 DoubleRow
    matmuls (cost-model 0.5 cycles/row, 256-deep contraction per
    instruction).  Full fp32-level accuracy is recovered with a hi/lo
    split: w = w_hi + w_lo, x = x_hi + x_lo (host-side, power-of-2
    rescaled so both splits stay in fp8's normal range), computing
    w_hi@x_hi (chunk-paired) + (w_hi@x_lo + w_lo@x_hi) (slot-paired in a
    single DoubleRow op).  The dropped w_lo@x_lo term is ~1e-3 relative.
  - All scale factors fold into free spots: the RoPE cos/sin tables carry
    the score scale, the softmax-denominator ones-matrix carries the V
    rescale, and the host divides the summed output by a single constant.
  - QKV accumulates entirely in PSUM (6 banks), no SBUF folding; weights
    and x stream once (fp8, half traffic).
  - Attention: flash-style transposed scores (fp32r), software-pipelined
    with a 3-tile scores/exp lookahead (plus a pre-ramp of 6 tiles emitted
    inside the preceding projection phase).  Causal handling skips
    above-diagonal key tiles, shrinks diagonal-tile matmuls to the live
    query range (>=256 free for fp32r full rate), and masks only the
    128-wide causal boundary block.  The softmax denominator comes from a
    4.0-matrix matmul; normalization and the attout hi/lo fp8 split run on
    DVE; RoPE is split DVE/GpSimd with per-engine scratch pools.
  - Output staged to bf16 in SBUF (ACT/DVE alternating copies, quad-batched
    DMAs; per-tile DMAs on the final row), host sums partials in fp32.
  - PE phase order: p0 p1 a0 p2 a1 w0 p3 a2 w1 a3 w2 w3 so RoPE, exp,
    attout finalization and output DMA all hide under PE work.  TimelineSim:
    316910 ns (baseline 468245 ns), PE busy 299 us (94%).
"""

import math
import os
import sys
import time

import numpy as np
import ml_dtypes

E4 = ml_dtypes.float8_e4m3fn


def _log(msg):
    if os.environ.get("KERNEL_QUIET"):
        return
    print(f"[kernel {time.strftime('%H:%M:%S')}] {msg}", file=sys.stderr, flush=True)


import concourse.bass as bass
import concourse.tile as tile
from concourse import bacc, mybir
from concourse.bass_utils import run_bass_kernel_spmd

S, D = 2048, 4096
H, H_KV, HD = 32, 8, 128
NCORES = 8
HPC = H // NCORES            # 4 Q heads per core
NSLAB = 6                    # q0..q3, k, v slabs of 128 cols
SQ = 512
NSQ = S // SQ                # 4
NKT = S // 128               # 16 key tiles
NDC = D // 128               # 32 contraction chunks
GRP = 8                      # chunks per DMA group
NG = NDC // GRP              # 4 groups
F32 = mybir.dt.float32
F32R = mybir.dt.float32r
F8 = mybir.dt.float8e4
BF16 = mybir.dt.bfloat16
DR = mybir.MatmulPerfMode.DoubleRow
Exp = mybir.ActivationFunctionType.Exp

WSC = 64.0                       # weight rescale for fp8 range
BETA = 1.0 / (WSC * 128.0 ** 0.25)   # rope cos/sin scale (per q and k)
CONES = 4.0                      # denominator matrix value -> attout = 16x true
OUT_DIV = (WSC / CONES) * WSC    # host divides summed output by this (1024)

_NC_CACHE = {}


def _build_nc():
    nc = bacc.Bacc(
        "TRN2", target_bir_lowering=False, debug=False, enable_asserts=False
    )
    w8 = nc.dram_tensor("w8", [D, 2 * NSLAB * 128], F8, kind="ExternalInput")
    # s-chunk-major: [NSQ * D, (lo, hi) * SQ] so each (g, s) load is 3-dim
    x8 = nc.dram_tensor("x8", [NSQ * D, 2 * SQ], F8, kind="ExternalInput")
    wor8 = nc.dram_tensor("wor8", [128, HPC * 2 * D], F8, kind="ExternalInput")
    cosq = nc.dram_tensor("cosq", [64, S], BF16, kind="ExternalInput")
    sinq = nc.dram_tensor("sinq", [64, S], BF16, kind="ExternalInput")
    diagm = nc.dram_tensor("diagm", [128, 256], BF16, kind="ExternalInput")
    ones4 = nc.dram_tensor("ones4", [128, 128], F32R, kind="ExternalInput")
    identd = nc.dram_tensor("identd", [128, 128], F32R, kind="ExternalInput")
    out = nc.dram_tensor("out", [S, D], BF16, kind="ExternalOutput")

    _log("emitting IR")
    with tile.TileContext(nc) as tc:
        _emit(tc, w8, x8, wor8, cosq, sinq, diagm, ones4, identd, out)
    _log("bacc compile")
    nc.compile()
    _log("bass module ready")
    return nc


def _emit(tc, w8, x8, wor8, cosq, sinq, diagm, ones4, identd, out):
    from contextlib import ExitStack

    nc = tc.nc
    WROW = 2 * NSLAB * 128       # 1536 fp8 cols per d-chunk of w8
    with ExitStack() as ctx:
        const = ctx.enter_context(tc.tile_pool(name="const", bufs=1))
        wres = ctx.enter_context(tc.tile_pool(name="wres", bufs=1))
        slabs = ctx.enter_context(tc.tile_pool(name="slabs", bufs=1))
        xpool = ctx.enter_context(tc.tile_pool(name="xpool", bufs=4))
        vtmp = ctx.enter_context(tc.tile_pool(name="vtmp", bufs=1))
        ptpool = ctx.enter_context(tc.tile_pool(name="ptpool", bufs=6))
        # separate rope scratch rings per engine: a shared ring would make
        # the Pool rope ops wait on DVE's tile releases (serializing them)
        tmppool = ctx.enter_context(tc.tile_pool(name="tmppool", bufs=4))
        tmppoolp = ctx.enter_context(tc.tile_pool(name="tmppoolp", bufs=4))
        recpool = ctx.enter_context(tc.tile_pool(name="recpool", bufs=1))
        atpool = ctx.enter_context(tc.tile_pool(name="atpool", bufs=1))
        a8pool = ctx.enter_context(tc.tile_pool(name="a8pool", bufs=2))
        wostg = ctx.enter_context(tc.tile_pool(name="wostg", bufs=3))
        ps8 = ctx.enter_context(tc.tile_pool(name="ps8", bufs=8, space="PSUM"))

        # resident weights
        w8t = wres.tile([128, NDC * WROW], F8)
        wrt = wres.tile([128, HPC * 2 * D], F8)

        # constants
        cosT = const.tile([128, S], BF16)
        sinT = const.tile([128, S], BF16)
        dmask = const.tile([128, 256], BF16)
        ones_t = const.tile([128, 128], F32R)
        ident = const.tile([128, 128], F32R)

        def load_consts():
            nc.sync.dma_start(cosT[0:64, :], cosq.ap())
            nc.sync.dma_start(cosT[64:128, :], cosq.ap())
            nc.sync.dma_start(sinT[0:64, :], sinq.ap())
            nc.sync.dma_start(sinT[64:128, :], sinq.ap())
            nc.sync.dma_start(dmask[:], diagm.ap())
            nc.sync.dma_start(ones_t[:], ones4.ap())

        def dma_w8(g, h0=0.0, h1=1.0):
            a = int((g + h0) * GRP * 128)
            b = int((g + h1) * GRP * 128)
            nc.sync.dma_start(
                w8t[:, a * WROW // 128 : b * WROW // 128].rearrange(
                    "p (c x) -> p c x", x=WROW
                ),
                w8.ap()[a:b, :].rearrange("(c p) x -> p c x", p=128),
            )

        # persistent q0..q3,k slabs (transposed [dim, seq]) + transposed V
        qkv = [
            [slabs.tile([128, SQ], F32R, name=f"qkv{s}_{i}") for i in range(5)]
            for s in range(NSQ)
        ]
        vt_s = [slabs.tile([128, SQ], F32R, name=f"vt{s}") for s in range(NSQ)]

        # ---- fused QKV projection: fp8 DoubleRow, PSUM-resident ----
        def proj(s, with_w=False, tail_hook=None):
            ps = [
                ps8.tile([128, SQ], F32, tag="ps", name=f"proj{s}_{nt}")
                for nt in range(NSLAB)
            ]
            started = [False] * NSLAB
            HG = GRP // 2        # chunks per x half-group tile
            for g in range(NG):
                for hf in range(2):
                    # the very first half-group streams w/x in single chunks
                    # so the PE starts as soon as the first chunk lands
                    first_hg = with_w and g == 0 and hf == 0
                    subs = (0.0, 0.25, 0.5, 1.0) if first_hg else (0.0, 1.0)
                    xg = xpool.tile([128, HG * 2 * SQ], F8, tag="x")
                    base = g * GRP + hf * HG     # absolute first chunk
                    for hh in range(len(subs) - 1):
                        c0 = int(subs[hh] * HG)
                        c1 = int(subs[hh + 1] * HG)
                        if with_w:
                            dma_w8(g, (hf * HG + c0) / GRP, (hf * HG + c1) / GRP)
                        nc.sync.dma_start(
                            xg[:, c0 * 2 * SQ : c1 * 2 * SQ].rearrange(
                                "p (c n) -> p c n", n=2 * SQ
                            ),
                            x8.ap()[
                                s * D + (base + c0) * 128 : s * D
                                + (base + c1) * 128,
                                :,
                            ].rearrange("(c p) x -> p c x", p=128),
                        )
                        wv_g = w8t[:, base * WROW : (base + HG) * WROW]
                        w_ch = wv_g.rearrange("p (c x) -> p c x", x=WROW)
                        w_sl = wv_g.rearrange(
                            "p (c t x) -> p c t x", t=2, x=NSLAB * 128
                        )
                        x_pr = xg[:].rearrange("p (c n) -> p c n", n=2 * SQ)
                        x_sl = xg[:].rearrange(
                            "p (c t n) -> p c t n", t=2, n=SQ
                        )
                        for nt in range(NSLAB):
                            co = nt * 128
                            # emit cross terms first (each needs only one
                            # chunk, letting the first matmuls start before
                            # the pair's second chunk lands)
                            last = g == NG - 1 and hf == 1 and c1 == HG
                            for dd in range(c0, c1):
                                # cross terms: (w_hi, w_lo) @ (x_lo, x_hi)
                                nc.tensor.matmul(
                                    ps[nt][:],
                                    w_sl[:, dd, :, co : co + 128],
                                    x_sl[:, dd],
                                    start=not started[nt],
                                    stop=False,
                                    perf_mode=DR,
                                )
                                started[nt] = True
                            for k in range(c0 // 2, (c1 + 1) // 2):
                                # hi@hi over chunk pair (2k, 2k+1) of the
                                # half-group; emitted once both chunks of
                                # the pair are covered
                                if 2 * k + 2 > c1:
                                    continue
                                nc.tensor.matmul(
                                    ps[nt][:],
                                    w_ch[:, 2 * k : 2 * k + 2, co : co + 128],
                                    x_pr[:, 2 * k : 2 * k + 2, SQ : 2 * SQ],
                                    start=False,
                                    stop=(last and 2 * k + 2 == HG),
                                    perf_mode=DR,
                                )
            # V first (its transpose is on the PE critical path), then the
            # next attention chunk's pre-ramp, then the remaining copies
            # spread over ACT/DVE so banks release in parallel and no
            # cross-engine ordering inversions appear.
            vs = vtmp.tile([128, SQ], F32R, tag="v")
            nc.vector.tensor_copy(vs[:], ps[5][:])
            if tail_hook is not None:
                tail_hook()
            for tt in range(4):
                tp = ps8.tile([128, 128], F32R, tag="ps", name=f"vtp{s}_{tt}")
                nc.tensor.transpose(
                    tp[:], vs[:, tt * 128 : (tt + 1) * 128], ident[:]
                )
                nc.scalar.copy(vt_s[s][:, tt * 128 : (tt + 1) * 128], tp[:])
            nc.scalar.copy(qkv[s][0][:], ps[0][:])
            nc.scalar.copy(qkv[s][1][:], ps[1][:])
            nc.vector.tensor_copy(qkv[s][2][:], ps[2][:])
            nc.vector.tensor_copy(qkv[s][3][:], ps[3][:])
            nc.scalar.copy(qkv[s][4][:], ps[4][:])

        def rope(s, dve_slabs=(4, 0)):
            cs_lo = cosT[0:64, s * SQ : (s + 1) * SQ]
            cs_hi = cosT[64:128, s * SQ : (s + 1) * SQ]
            sn_lo = sinT[0:64, s * SQ : (s + 1) * SQ]
            sn_hi = sinT[64:128, s * SQ : (s + 1) * SQ]
            for nt in (4, 0, 1, 2, 3):
                dve = nt in dve_slabs
                eng = nc.vector if dve else nc.gpsimd
                pool = tmppool if dve else tmppoolp
                tl = qkv[s][nt]
                lo = tl[0:64, :]
                hi = tl[64:128, :]
                m1 = pool.tile([64, SQ], F32, tag="t")
                m2 = pool.tile([64, SQ], F32, tag="t")
                m3 = pool.tile([64, SQ], F32, tag="t")
                m4 = pool.tile([64, SQ], F32, tag="t")
                eng.tensor_mul(m1[:], lo, cs_lo)
                eng.tensor_mul(m2[:], hi, sn_hi)
                eng.tensor_mul(m3[:], lo, sn_lo)
                eng.tensor_mul(m4[:], hi, cs_hi)
                eng.tensor_sub(hi, m1[:], m2[:])   # rotated low half
                eng.tensor_add(lo, m3[:], m4[:])   # rotated high half

        def ktile(t):
            return qkv[t // 4][4][:, (t % 4) * 128 : (t % 4) * 128 + 128]

        def vtile(t):
            return vt_s[t // 4][:, (t % 4) * 128 : (t % 4) * 128 + 128]

        attout8 = {}

        # ---- attention (flash, transposed scores, causal block skip) ----
        # software-pipelined: the scores matmul + exp for tile i+L issue
        # before the av/den matmuls of tile i, so the PE never waits on the
        # ACT exp latency.
        def att_make(c, L=3, npre=6):
            """Returns (pre, rest): pre emits the first `npre` sc/exp chains
            (callable from inside the preceding proj phase, using the spare
            PSUM banks); rest emits everything else."""
            ntiles = 4 * c + 4
            avden = {}
            state = {"a8v": None}
            pend = []
            stream = [(h, t) for h in range(HPC) for t in range(ntiles)]

            def finalize(h):
                a8v = state["a8v"]
                av, den = avden.pop(h)
                rec = recpool.tile([128, SQ], F32, tag="rec")
                nc.vector.reciprocal(rec[:], den[:])
                t_f = atpool.tile([128, SQ], F32, tag="t")
                nc.vector.tensor_mul(t_f[:], av[:], rec[:])
                nc.vector.tensor_copy(a8v[:, h, 0, :], t_f[:])  # hi (fp8 cast)
                nc.vector.tensor_sub(a8v[:, h, 1, :], t_f[:], a8v[:, h, 0, :])

            def emit_sc(h, t):
                j = t - 4 * c
                # diagonal tiles: queries below the causal frontier are all
                # masked; shrink the moving free dim (kept >= 256 for fp32r)
                qo = 0 if j < 1 else (128 if j == 1 else 256)
                fr = SQ - qo
                sc = ps8.tile([128, fr], F32, tag="ps", name=f"sc{h}_{c}_{t}")
                nc.tensor.matmul(
                    sc[:], ktile(t), qkv[c][h][:, qo:SQ], start=True, stop=True
                )
                if j >= 0:
                    # only the 128-wide causal boundary block needs masking
                    # (plus one fully-masked block for j=3 whose q-slice
                    # starts below the frontier)
                    if j == 3:
                        nc.vector.tensor_add(
                            sc[:, 0:256], sc[:, 0:256], dmask[:, 0:256]
                        )
                    else:
                        nc.vector.tensor_add(
                            sc[:, 0:128], sc[:, 0:128], dmask[:, 128:256]
                        )
                pt = ptpool.tile([128, fr], F32R, tag="pt")
                nc.scalar.activation(pt[:], sc[:], Exp)
                pend.append((h, t, qo, pt))

            def pre():
                for h, t in stream[:npre]:
                    emit_sc(h, t)

            def rest():
                a8 = a8pool.tile(
                    [128, HPC * 2 * SQ], F8, tag="a8", name=f"a8_{c}"
                )
                attout8[c] = a8
                state["a8v"] = a8[:].rearrange("p (h t n) -> p h t n", h=HPC, t=2)
                for h, t in stream[npre:]:
                    emit_sc(h, t)
                    if len(pend) > L:
                        emit_avden(c, ntiles, avden, pend.pop(0), finalize)
                while pend:
                    emit_avden(c, ntiles, avden, pend.pop(0), finalize)

            return pre, rest

        def emit_avden(c, ntiles, avden, item, finalize):
            h, t, qo, pt = item
            if t == 0:
                avden[h] = (
                    ps8.tile([128, SQ], F32, tag="ps", name=f"av{h}_{c}"),
                    ps8.tile([128, SQ], F32, tag="ps", name=f"den{h}_{c}"),
                )
            av, den = avden[h]
            last = t == ntiles - 1
            nc.tensor.matmul(
                den[:, qo:SQ], ones_t[:], pt[:], start=(t == 0), stop=last
            )
            nc.tensor.matmul(
                av[:, qo:SQ], vtile(t), pt[:], start=(t == 0), stop=last
            )
            if last:
                finalize(h)

        # ---- wo projection: fp8 DoubleRow, bf16 staged output ----
        def wo(c, last=False):
            a8 = attout8.pop(c)
            a8v = a8[:].rearrange("p (h t n) -> p h t n", h=HPC, t=2)
            wrv = wrt[:].rearrange("p (h t n) -> p h t n", h=HPC, t=2)
            for m in range(4):
                mo = m * 128
                # final row of the kernel drains per-tile so the last DMA is
                # small (shorter end-of-kernel tail)
                fine = last and m == 3
                for jq in range(2):
                    st = wostg.tile([128, 4 * SQ], BF16, tag="st")
                    for jj in range(4):
                        j = jq * 4 + jj
                        po = ps8.tile(
                            [128, SQ], F32, tag="ps", name=f"po{c}_{m}_{j}"
                        )
                        for p in range(HPC // 2):
                            # hi@hi over head pair (2p, 2p+1)
                            nc.tensor.matmul(
                                po[:],
                                a8v[:, 2 * p : 2 * p + 2, 0, mo : mo + 128],
                                wrv[:, 2 * p : 2 * p + 2, 1, j * SQ : (j + 1) * SQ],
                                start=(p == 0),
                                stop=False,
                                perf_mode=DR,
                            )
                        for h in range(HPC):
                            # cross: (a_hi, a_lo) @ (wo_lo, wo_hi)
                            nc.tensor.matmul(
                                po[:],
                                a8v[:, h, :, mo : mo + 128],
                                wrv[:, h, :, j * SQ : (j + 1) * SQ],
                                start=False,
                                stop=(h == HPC - 1),
                                perf_mode=DR,
                            )
                        # GPSIMD cannot read PSUM; alternate ACT/DVE
                        eng = (nc.scalar.copy, nc.vector.tensor_copy)[
                            (m * 8 + j) % 2
                        ]
                        eng(st[:, jj * SQ : (jj + 1) * SQ], po[:])
                        if fine:
                            nc.sync.dma_start(
                                out.ap()[
                                    (4 * c + m) * 128 : (4 * c + m + 1) * 128,
                                    j * SQ : (j + 1) * SQ,
                                ],
                                st[:, jj * SQ : (jj + 1) * SQ],
                            )
                    if not fine:
                        nc.sync.dma_start(
                            out.ap()[
                                (4 * c + m) * 128 : (4 * c + m + 1) * 128,
                                jq * 4 * SQ : (jq + 1) * 4 * SQ,
                            ],
                            st[:],
                        )

        # ---- schedule ----
        # ident MUST be emitted before proj(0): its V-transposes read it,
        # and a read emitted before the write would consume garbage
        nc.sync.dma_start(ident[:], identd.ap())
        proj(0, with_w=True)
        load_consts()
        rope(0)
        a0_pre, a0_rest = att_make(0)
        proj(1, tail_hook=a0_pre)
        a0_rest()
        rope(1)
        a1_pre, a1_rest = att_make(1)
        proj(2, tail_hook=a1_pre)
        nc.sync.dma_start(wrt[:], wor8.ap())
        a1_rest()
        wo(0)
        rope(2)
        a2_pre, a2_rest = att_make(2)
        proj(3, tail_hook=a2_pre)
        a2_rest()
        wo(1)
        rope(3)
        a3_pre, a3_rest = att_make(3)
        a3_pre()
        a3_rest()
        wo(2)
        wo(3, last=True)


def _host_prep(x, wq, wk, wv, wo, freqs_cos, freqs_sin):
    """Build the 8 per-core input maps (fp8 hi/lo splits, rescaled)."""
    perm = np.concatenate([np.arange(0, HD, 2), np.arange(1, HD, 2)])
    xt = np.ascontiguousarray(x.reshape(S, D).T)
    xhi = xt.astype(E4)
    xlo = (xt - xhi.astype(np.float32)).astype(E4)
    # s-chunk-major, (lo, hi) slots: [NSQ, D, 2, SQ] -> [NSQ * D, 2 * SQ]
    x8 = np.ascontiguousarray(
        np.stack(
            [
                xlo.reshape(D, NSQ, SQ).transpose(1, 0, 2),
                xhi.reshape(D, NSQ, SQ).transpose(1, 0, 2),
            ],
            axis=2,
        ).reshape(NSQ * D, 2 * SQ)
    )
    cosq = np.ascontiguousarray((freqs_cos.T.astype(np.float32) * BETA).astype(ml_dtypes.bfloat16))
    sinq = np.ascontiguousarray((freqs_sin.T.astype(np.float32) * BETA).astype(ml_dtypes.bfloat16))
    kk = np.arange(128)[:, None]
    qq = np.arange(128)[None, :]
    tri = np.where(kk <= qq, 0.0, -1e9).astype(np.float32)
    diagm = np.concatenate(
        [np.full((128, 128), -1e9, np.float32), tri], axis=1
    ).astype(ml_dtypes.bfloat16)
    ones4 = np.full((128, 128), CONES, np.float32)
    ident = np.eye(128, dtype=np.float32)

    in_maps = []
    for c in range(NCORES):
        wq_c = (
            wq[:, (HPC * c) * HD : (HPC * c + HPC) * HD]
            .reshape(D, HPC, HD)[:, :, perm]
            .reshape(D, HPC * HD)
        )
        wk_c = wk[:, c * HD : (c + 1) * HD][:, perm]
        wv_c = wv[:, c * HD : (c + 1) * HD]
        wcat = np.concatenate([wq_c, wk_c, wv_c], axis=1) * WSC  # [D, 768]
        whi = wcat.astype(E4)
        wlo = (wcat - whi.astype(np.float32)).astype(E4)
        w8 = np.ascontiguousarray(
            np.stack([whi, wlo], axis=1).reshape(D, 2 * NSLAB * 128)
        )  # (hi, lo)
        wo_c = (
            wo[(HPC * c) * HD : (HPC * c + HPC) * HD, :].reshape(HPC, 128, D)
            * WSC
        )
        wo_hd = wo_c.transpose(1, 0, 2)  # [128 hd, HPC, D]
        whi_o = wo_hd.astype(E4)
        wlo_o = (wo_hd - whi_o.astype(np.float32)).astype(E4)
        wor8 = np.ascontiguousarray(
            np.stack([wlo_o, whi_o], axis=2).reshape(128, HPC * 2 * D)
        )  # (lo, hi)
        in_maps.append(
            {
                "w8": w8,
                "x8": x8,
                "wor8": wor8,
                "cosq": cosq,
                "sinq": sinq,
                "diagm": diagm,
                "ones4": ones4,
                "identd": ident,
            }
        )
    return in_maps


def _numpy_fallback(x, wq, wk, wv, wo, freqs_cos, freqs_sin, mask):
    """Exact reference math in numpy (used only for non-causal masks)."""
    bsz = x.shape[0]
    n_rep = H // H_KV
    xq = (x.reshape(-1, D) @ wq).reshape(bsz, S, H, HD)
    xk = (x.reshape(-1, D) @ wk).reshape(bsz, S, H_KV, HD)
    xv = (x.reshape(-1, D) @ wv).reshape(bsz, S, H_KV, HD)

    def rope(t):
        t0, t1 = t[..., 0::2], t[..., 1::2]
        c = freqs_cos[None, :, None, :]
        s = freqs_sin[None, :, None, :]
        o0 = t0 * c - t1 * s
        o1 = t0 * s + t1 * c
        return np.stack([o0, o1], axis=-1).reshape(t.shape)

    xq, xk = rope(xq), rope(xk)
    keys = np.repeat(xk, n_rep, axis=2)
    values = np.repeat(xv, n_rep, axis=2)
    scores = np.einsum("bqhd,bkhd->bhqk", xq, keys) / math.sqrt(HD)
    scores = scores + mask[:, :, -S:, -S:]
    scores = scores - scores.max(axis=-1, keepdims=True)
    e = np.exp(scores)
    attn = e / e.sum(axis=-1, keepdims=True)
    o = np.einsum("bhqk,bkhd->bqhd", attn, values).reshape(bsz, S, H * HD)
    return (o @ wo).astype(np.float32)


def kernel(**inputs):
    x = np.asarray(inputs["x"], dtype=np.float32)
    wq = np.asarray(inputs["wq"], dtype=np.float32)
    wk = np.asarray(inputs["wk"], dtype=np.float32)
    wv = np.asarray(inputs["wv"], dtype=np.float32)
    wo = np.asarray(inputs["wo"], dtype=np.float32)
    fc = np.asarray(inputs["freqs_cos"], dtype=np.float32)
    fs = np.asarray(inputs["freqs_sin"], dtype=np.float32)
    mask = np.asarray(inputs["mask"], dtype=np.float32)

    causal = np.triu(np.full((S, S), -1e9, dtype=np.float32), k=1)[None, None]
    if x.shape != (1, S, D) or not np.array_equal(mask, causal):
        return _numpy_fallback(x, wq, wk, wv, wo, fc, fs, mask)

    if "nc" not in _NC_CACHE:
        _NC_CACHE["nc"] = _build_nc()
    nc = _NC_CACHE["nc"]
    in_maps = _host_prep(x[0], wq, wk, wv, wo, fc, fs)
    _log("launching on 8 cores (compile on first call + transfers)")
    res = run_bass_kernel_spmd(nc, in_maps, core_ids=list(range(NCORES)))
    _log("run complete")
    full = np.zeros((S, D), np.float32)
    for r in res.results:
        full += r["out"].astype(np.float32)
    full /= OUT_DIV
    return full.reshape(1, S, D)
